# revision 33
# baseline (speedup 1.0000x reference)
"""Graphormer-expert GNN kernel for 8 Trainium2 NeuronCores.

Strategy (matches the sharding hint): nodes are partitioned 8 x 6250 (graph
parallel); each core owns the edges whose *target* falls in its shard, so the
scatter-softmax is core-local.  Per layer each core computes LN + projections
for its own nodes, k|v rows (bf16) are exchanged with an AllGather, and
per-edge k/v rows are fetched with SWDGE dma_gather (int16 indices; source
table split in two 25088-row buckets; padding indices are -1 so the Q7
descriptor generator trims them).  Per-edge softmax runs without
max-subtraction (|alpha| << 1 for this model); the q-row broadcast and the
segment sums (softmax denominator + message aggregation) run on the
TensorEngine with host-precomputed fp8 one-hot matrices (exact), accumulating
each target-block's edge tiles in PSUM.  The softmax division is commuted
past the segment sum and applied per node.  k-bias drops out exactly (softmax
shift invariance); v-bias is folded in after aggregation.  ACT runs only
copies + Exp (sigmoid via exp, layernorm rsqrt via DVE Newton iteration) so
the activation table is loaded once.
"""

import sys

sys.path.insert(0, "/opt/trn_rl_repo")

import numpy as np

N, IN_DIM, D, H, L, E, MAX_DEG = 50000, 128, 128, 16, 3, 800000, 512
C = D // H
P = 128
NCORES = 8
NSH = N // NCORES            # 6250 nodes per core
NBLK = (NSH + P - 1) // P    # 49 target blocks per core
NPAD = NBLK * P              # 6272 padded rows per core
CH_BLK = [0, 12, 24, 36, 48, 49]      # AllGather chunk boundaries (blocks)
CH_ROW = [b * P for b in CH_BLK]       # per-core row boundaries
NBUK0 = 8 * CH_ROW[2]                  # bucket 0 = chunks 0-1 = 24576 rows
NBUK1 = 8 * (CH_ROW[-1] - CH_ROW[2])   # bucket 1 = chunks 2.. = 25600 rows

RSQRT_MAGIC = 0x5F3759DF


def _bf16(a):
    import ml_dtypes

    return np.asarray(a, dtype=ml_dtypes.bfloat16)


def _fp8(a):
    import ml_dtypes

    return np.asarray(a, dtype=ml_dtypes.float8_e4m3)


def _wrap_idx16(idx, pad_to=None, fill=0):
    """int16 idx array -> [128, n/16] wrapped (j -> [j%16, j//16]) and
    replicated across the 8 gpsimd cores' 16-partition groups."""
    n = len(idx) if pad_to is None else pad_to
    assert n % 16 == 0
    a = np.full(n, fill, dtype=np.int16)
    a[: len(idx)] = idx.astype(np.int16)
    w = a.reshape(n // 16, 16).T  # [16, n/16]
    return np.tile(w, (8, 1))  # [128, n/16]


def _preprocess(x, edge_index):
    """Host-side integer/index preprocessing + per-core shard arrays."""
    src = np.asarray(edge_index[0], dtype=np.int64)
    tgt = np.asarray(edge_index[1], dtype=np.int64)

    # degrees (int) for the centrality embeddings (applied as one-hot matmuls)
    idg = np.clip(np.bincount(tgt, minlength=N), 0, MAX_DEG)
    odg = np.clip(np.bincount(src, minlength=N), 0, MAX_DEG)
    dmax = int(max(idg.max(), odg.max()))
    KROWS = 64 if dmax < 64 else 128
    assert dmax < KROWS

    # global row in the chunked-AllGather kv table of node g: chunk-major,
    # then core, then row-within-chunk
    sc = src // NSH
    sr = src % NSH
    chv = np.searchsorted(np.asarray(CH_ROW), sr, side="right") - 1
    nch = len(CH_ROW) - 1
    chw = np.asarray([CH_ROW[i + 1] - CH_ROW[i] for i in range(nch)])
    ch_base8 = np.asarray([8 * CH_ROW[i] for i in range(nch)])
    kv_row = ch_base8[chv] + sc * chw[chv] + (sr - np.asarray(CH_ROW)[chv])
    bucket = (kv_row >= NBUK0).astype(np.int64)
    src_loc = kv_row - bucket * NBUK0  # int16-safe (< 25600)

    # first pass: find the max (block,bucket) run length across all cores
    run_max = 0
    per_core = []
    for c in range(NCORES):
        m = (tgt // NSH) == c
        cs, ct, cb, csl = src[m], tgt[m] - c * NSH, bucket[m], src_loc[m]
        blk = ct // P
        cnt = np.bincount(blk * 2 + cb, minlength=NBLK * 2)
        run_max = max(run_max, cnt.max())
        per_core.append((cs, ct, cb, csl, blk))
    trun = int((run_max + P - 1) // P)  # tiles per (block,bucket) run
    nrun = trun * P
    KIB = trun * 16                     # idx bytes per partition per run
    EDB = KIB + 2 * trun * P            # + sg fp8 + st fp8 bytes

    ncalls = (trun + GBATCH - 1) // GBATCH
    min_run = min(
        int(np.bincount(blk * 2 + cb, minlength=NBLK * 2).min())
        for (_, _, cb, _, blk) in per_core
    )
    cores = []
    for c in range(NCORES):
        cs, ct, cb, csl, blk = per_core[c]
        order = np.lexsort((ct, cb, blk))
        cs, ct, cb, csl, blk = (a[order] for a in (cs, ct, cb, csl, blk))

        edata = np.zeros((P, NBLK * 2 * EDB), dtype=np.uint8)
        gcnt = np.zeros((NBLK * 2, ncalls), dtype=np.int32)
        for k in range(NBLK):
            for b in range(2):
                sel = (cb == b) & (blk == k)
                n_e = int(sel.sum())
                for ci, i0 in enumerate(range(0, trun, GBATCH)):
                    i1 = min(i0 + GBATCH, trun)
                    gcnt[k * 2 + b, ci] = max(0, min(n_e - i0 * P, (i1 - i0) * P))
                ki = _wrap_idx16(csl[sel], pad_to=nrun, fill=-1)  # [128, trun*8]
                tl = ct[sel] - k * P  # 0..127 col within the block
                ee = np.arange(n_e)
                S = np.zeros((P, nrun), dtype=np.float32)   # [e%128, t*128+tl]
                S[ee % P, (ee // P) * P + tl] = 1.0
                ST = np.zeros((P, nrun), dtype=np.float32)  # [tl, e]
                ST[tl, ee] = 1.0
                off = (k * 2 + b) * EDB
                edata[:, off : off + KIB] = ki.view(np.uint8)
                edata[:, off + KIB : off + KIB + nrun] = _fp8(S).view(np.uint8)
                edata[:, off + KIB + nrun : off + EDB] = _fp8(ST).view(np.uint8)

        cidg = np.pad(idg[c * NSH : (c + 1) * NSH], (0, NPAD - NSH))
        codg = np.pad(odg[c * NSH : (c + 1) * NSH], (0, NPAD - NSH))
        degoh = np.zeros((KROWS, NBLK * 2 * P), dtype=np.float32)
        for k in range(NBLK):
            nodes = np.arange(k * P, (k + 1) * P)
            degoh[cidg[nodes], k * 2 * P + np.arange(P)] = 1.0
            degoh[codg[nodes], k * 2 * P + P + np.arange(P)] = 1.0

        cores.append(
            dict(
                edata=edata,
                gcnt=np.broadcast_to(gcnt.reshape(1, -1), (P, NBLK * 2 * ncalls)).copy(),
                degoh=_fp8(degoh),
                x=np.pad(
                    np.asarray(x[c * NSH : (c + 1) * NSH], dtype=np.float32),
                    ((0, NPAD - NSH), (0, 0)),
                ),
            )
        )
    return cores, trun, EDB, min_run, KROWS


import os as _os

PROBE_NO_COLLECTIVE = bool(int(_os.environ.get("KB_NOCOLL", "0")))
ABL_NOEDGE = bool(int(_os.environ.get("KB_NOEDGE", "0")))   # skip edge interior

GBATCH = int(_os.environ.get("KB_GBATCH", "5"))   # kv gather tiles per call
EBATCH = int(_os.environ.get("KB_EBATCH", "3"))   # emb gather blocks per call
EPBUFS = int(_os.environ.get("KB_EPBUFS", "4"))   # edge pool depth
GPBUFS = int(_os.environ.get("KB_GPBUFS", "8"))   # gather pool depth (ed+kvg)


def _build(trun, EDB, min_run=0, KROWS=64):
    from concourse import bass, mybir
    import concourse.tile as tile
    from concourse.bacc import Bacc
    from concourse.masks import make_identity

    dt = mybir.dt
    AX = mybir.AxisListType
    OP = mybir.AluOpType
    AF = mybir.ActivationFunctionType

    KIB = trun * 16
    nrun = trun * P

    nc = Bacc(None, target_bir_lowering=False, debug=False, num_devices=NCORES,
              num_swdge_queues=4)
    qctr = [0]

    def _nextq():
        qctr[0] = (qctr[0] + 1) % 4
        return qctr[0]

    # ---- parameters (per core) -------------------------------------------
    xin = nc.declare_dram_parameter("x", [NPAD, D], dt.float32, isOutput=False)
    deg_p = nc.declare_dram_parameter("degoh", [KROWS, NBLK * 2 * P], dt.float8e4, isOutput=False)
    eio_p = nc.declare_dram_parameter("embio", [KROWS, 2 * D], dt.bfloat16, isOutput=False)
    win_p = nc.declare_dram_parameter("win", [D, D], dt.bfloat16, isOutput=False)
    bin_p = nc.declare_dram_parameter("bin", [P, D], dt.float32, isOutput=False)
    wcat_p = nc.declare_dram_parameter("wcat", [D, L * 4 * D], dt.bfloat16, isOutput=False)
    bcat_p = nc.declare_dram_parameter("bcat", [P, L * 2 * D], dt.float32, isOutput=False)
    bvp_p = nc.declare_dram_parameter("bvp", [P, L * D], dt.float32, isOutput=False)
    lnp_p = nc.declare_dram_parameter("lnp", [P, L * 2 * D], dt.float32, isOutput=False)
    fnp_p = nc.declare_dram_parameter("fnp", [P, 2 * D], dt.float32, isOutput=False)
    wb_p = nc.declare_dram_parameter("wbeta", [P, L * 2 * D], dt.float32, isOutput=False)
    ed_p = nc.declare_dram_parameter("edata", [P, NBLK * 2 * EDB], dt.uint8, isOutput=False)
    NC_G = (trun + GBATCH - 1) // GBATCH
    gc_p = nc.declare_dram_parameter("gcnt", [P, NBLK * 2 * NC_G], dt.int32, isOutput=False)
    out_p = nc.declare_dram_parameter("out", [NSH, D], dt.float32, isOutput=True)

    # ---- DRAM scratch -----------------------------------------------------
    kvb = nc.dram_tensor("kv_bounce", [NPAD, 2 * D], dt.bfloat16)
    kvfs = [
        nc.dram_tensor(f"kv_full{i}", [NCORES * NPAD, 2 * D], dt.bfloat16, addr_space="Shared")
        for i in range(2)
    ]

    with tile.TileContext(nc) as tc:
        with (
            tc.tile_pool(name="persist", bufs=1) as pp,
            tc.tile_pool(name="wtiles", bufs=1) as wp,
            tc.tile_pool(name="work", bufs=1) as kp,
            tc.tile_pool(name="small", bufs=3) as sp,
            tc.tile_pool(name="edge", bufs=EPBUFS) as ep,
            tc.tile_pool(name="gath", bufs=GPBUFS) as gp,
            tc.tile_pool(name="psA", bufs=1, space="PSUM") as psA,
            tc.tile_pool(name="psB", bufs=2, space="PSUM") as psB,
            tc.tile_pool(name="psC", bufs=2, space="PSUM") as psC,
            tc.tile_pool(name="psQ", bufs=2, space="PSUM") as psQ,
        ):
            # persistent state
            h = pp.tile([P, NBLK, D], dt.float32, tag="h")
            xr = pp.tile([P, NBLK, D], dt.bfloat16, tag="xr")
            qsb = pp.tile([P, NBLK, D], dt.bfloat16, tag="qsb")

            ident = wp.tile([P, P], dt.bfloat16, tag="ident")
            make_identity(nc, ident[:])
            win = wp.tile([D, D], dt.bfloat16, tag="win")
            nc.sync.dma_start(win[:], win_p.ap())
            bin_t = wp.tile([P, D], dt.float32, tag="bin")
            nc.sync.dma_start(bin_t[:], bin_p.ap())
            wcat = wp.tile([D, L, 4 * D], dt.bfloat16, tag="wcat")
            nc.sync.dma_start(wcat[:], wcat_p.ap())
            bcat = wp.tile([P, L, 2 * D], dt.float32, tag="bcat")
            nc.sync.dma_start(bcat[:], bcat_p.ap())
            bvt = wp.tile([P, L, D], dt.float32, tag="bvt")
            nc.sync.dma_start(bvt[:], bvp_p.ap())
            lnp = wp.tile([P, L, 2 * D], dt.float32, tag="lnp")
            nc.sync.dma_start(lnp[:], lnp_p.ap())
            fnp = wp.tile([P, 2 * D], dt.float32, tag="fnp")
            nc.sync.dma_start(fnp[:], fnp_p.ap())
            wb = wp.tile([P, L, 2 * D], dt.float32, tag="wb")
            nc.sync.dma_start(wb[:], wb_p.ap())
            gct = wp.tile([P, NBLK * 2 * NC_G], dt.int32, tag="gct")
            nc.sync.dma_start(gct[:], gc_p.ap())
            eio = wp.tile([KROWS, 2 * D], dt.bfloat16, tag="eio")
            nc.sync.dma_start(eio[:], eio_p.ap())
            deg = wp.tile([KROWS, NBLK, 2 * P], dt.float8e4, tag="deg")
            nc.sync.dma_start(deg[:], deg_p.ap())
            gregs = [nc.gpsimd.alloc_register(f"gcnt_reg{i}") for i in range(8)]
            gregc = [0]

            def _rsqrt(rs, ve):
                """rs = 1/sqrt(ve) via bit-hack seed + 2 Newton iterations.
                rs, ve: [P, 1] f32 tiles (DVE only — no ACT table)."""
                iv = sp.tile([P, 1], dt.int32, tag="nw_i")
                nc.vector.tensor_scalar(
                    out=iv[:], in0=ve[:].bitcast(dt.int32), scalar1=1,
                    scalar2=None, op0=OP.logical_shift_right,
                )
                nc.vector.tensor_scalar(
                    out=iv[:], in0=iv[:], scalar1=-1, scalar2=RSQRT_MAGIC,
                    op0=OP.mult, op1=OP.add,
                )
                y = iv[:].bitcast(dt.float32)
                t = sp.tile([P, 1], dt.float32, tag="nw_t")
                cur = y
                for it in range(2):
                    nxt = rs[:] if it == 1 else t[:]
                    nc.vector.tensor_tensor(out=nxt, in0=cur, in1=cur, op=OP.mult)
                    nc.vector.tensor_tensor(out=nxt, in0=nxt, in1=ve[:], op=OP.mult)
                    nc.vector.tensor_scalar(
                        out=nxt, in0=nxt, scalar1=-0.5, scalar2=1.5,
                        op0=OP.mult, op1=OP.add,
                    )
                    nc.vector.tensor_tensor(out=nxt, in0=cur, in1=nxt, op=OP.mult)
                    cur = nxt

            def _ln_to(hb, t, scale_ap, bias_ap):
                """hb[P, D] (bf16) = LN(h[:, t, :]) * scale + bias."""
                stats = sp.tile([P, 6], dt.float32, tag="bst")
                nc.vector.bn_stats(stats[:], h[:, t, :])
                mv = sp.tile([P, 2], dt.float32, tag="mv")
                nc.vector.bn_aggr(mv[:], stats[:])
                ve = sp.tile([P, 1], dt.float32, tag="ve")
                nc.vector.tensor_scalar_add(ve[:], mv[:, 1:2], 1e-5)
                rs = sp.tile([P, 1], dt.float32, tag="rs")
                _rsqrt(rs, ve)
                hf = sp.tile([P, D], dt.float32, tag="hf")
                nc.vector.tensor_tensor(
                    out=hf[:], in0=h[:, t, :],
                    in1=mv[:, 0:1].to_broadcast([P, D]), op=OP.subtract,
                )
                nc.vector.scalar_tensor_tensor(
                    out=hb[:], in0=hf[:], scalar=rs[:], in1=scale_ap,
                    op0=OP.mult, op1=OP.mult,
                )
                nc.vector.tensor_tensor(out=hb[:], in0=hb[:], in1=bias_ap, op=OP.add)

            def _lnproj_block(t, layer):
                """LN h[:,t] (lnp[layer]) -> proj (wcat[layer]) -> kvb/qsb/xr."""
                hb = sp.tile([P, D], dt.bfloat16, tag="hb")
                _ln_to(hb, t, lnp[:, layer, 0:D], lnp[:, layer, D : 2 * D])
                pT = psA.tile([P, P], dt.bfloat16, tag="pT")
                nc.tensor.transpose(out=pT[:], in_=hb[:], identity=ident[:])
                hnTt = sp.tile([P, D], dt.bfloat16, tag="hnTt")
                nc.scalar.copy(hnTt[:], pT[:])
                ps = psB.tile([P, 4 * D], dt.float32, tag="ps")
                nc.tensor.matmul(
                    out=ps[:], lhsT=hnTt[:], rhs=wcat[:, layer, :],
                    start=True, stop=True,
                )
                kvq = sp.tile([P, 2 * D], dt.bfloat16, tag="kvq")
                nc.scalar.copy(kvq[:], ps[:, 0 : 2 * D])
                nc.vector.scalar_tensor_tensor(
                    out=qsb[:, t, :], in0=ps[:, 2 * D : 3 * D], scalar=1.0,
                    in1=bcat[:, layer, 0:D], op0=OP.mult, op1=OP.add,
                )
                nc.vector.scalar_tensor_tensor(
                    out=xr[:, t, :], in0=ps[:, 3 * D : 4 * D], scalar=1.0,
                    in1=bcat[:, layer, D : 2 * D], op0=OP.mult, op1=OP.add,
                )
                nc.scalar.dma_start(kvb.ap()[t * P : (t + 1) * P, :], kvq[:])
                if t + 1 in CH_BLK:
                    ch = CH_BLK.index(t + 1) - 1
                    r0, r1 = CH_ROW[ch], CH_ROW[ch + 1]
                    kvf_l = kvfs[layer % 2]
                    if PROBE_NO_COLLECTIVE:
                        nc.gpsimd.dma_start(
                            out=kvf_l.ap()[8 * r0 : 8 * r0 + (r1 - r0), :],
                            in_=kvb.ap()[r0:r1, :],
                        )
                    else:
                        nc.gpsimd.collective_compute(
                            "AllGather",
                            OP.bypass,
                            replica_groups=[list(range(NCORES))],
                            ins=[kvb.ap()[r0:r1, :].opt()],
                            outs=[kvf_l.ap()[8 * r0 : 8 * r1, :].opt()],
                        )

            def _final_block(t):
                """Final LN on h[:,t] -> out DMA."""
                ot = sp.tile([P, D], dt.float32, tag="ot")
                stats = sp.tile([P, 6], dt.float32, tag="bst")
                nc.vector.bn_stats(stats[:], h[:, t, :])
                mv = sp.tile([P, 2], dt.float32, tag="mv")
                nc.vector.bn_aggr(mv[:], stats[:])
                ve = sp.tile([P, 1], dt.float32, tag="ve")
                nc.vector.tensor_scalar_add(ve[:], mv[:, 1:2], 1e-5)
                rs = sp.tile([P, 1], dt.float32, tag="rs")
                _rsqrt(rs, ve)
                nc.vector.tensor_tensor(
                    out=ot[:], in0=h[:, t, :],
                    in1=mv[:, 0:1].to_broadcast([P, D]), op=OP.subtract,
                )
                nc.vector.scalar_tensor_tensor(
                    out=ot[:], in0=ot[:], scalar=rs[:], in1=fnp[:, 0:D],
                    op0=OP.mult, op1=OP.mult,
                )
                nc.vector.tensor_tensor(
                    out=ot[:], in0=ot[:], in1=fnp[:, D : 2 * D], op=OP.add
                )
                lo = t * P
                hi = min((t + 1) * P, NSH)
                if hi > lo:
                    nc.scalar.dma_start(out_p.ap()[lo:hi, :], ot[0 : hi - lo, :])

            # zero the kvg pool buffers once (trimmed gathers leave stale
            # bytes behind; first use must not see NaN bit patterns)
            for _ in range(GPBUFS):
                z = gp.tile([P, trun, 2 * D], dt.bfloat16, tag="kvg")
                nc.vector.memset(z[:], 0.0)

            # ---- phase 0 (fused with layer-0 LN+proj):
            # h = x @ W_in + b_in + emb_in[idg] + emb_out[odg], the embedding
            # gathers expressed as one-hot matmuls accumulated in PSUM
            for t in range(NBLK):
                xt = sp.tile([P, D], dt.float32, tag="xt")
                nc.sync.dma_start(xt[:], xin.ap()[t * P : (t + 1) * P, :])
                xb = sp.tile([P, D], dt.bfloat16, tag="xb")
                nc.vector.tensor_copy(xb[:], xt[:])
                pT = psA.tile([P, P], dt.bfloat16, tag="pT")
                nc.tensor.transpose(out=pT[:], in_=xb[:], identity=ident[:])
                xTb = sp.tile([P, D], dt.bfloat16, tag="xTb")
                nc.scalar.copy(xTb[:], pT[:])
                ph = psB.tile([P, 4 * D], dt.float32, tag="ps")
                nc.tensor.matmul(out=ph[:, 0:D], lhsT=xTb[:], rhs=win[:], start=True, stop=False)
                nc.tensor.matmul(
                    out=ph[:, 0:D], lhsT=deg[:, t, 0:P], rhs=eio[:, 0:D],
                    start=False, stop=False,
                )
                nc.tensor.matmul(
                    out=ph[:, 0:D], lhsT=deg[:, t, P : 2 * P], rhs=eio[:, D : 2 * D],
                    start=False, stop=True,
                )
                nc.vector.scalar_tensor_tensor(
                    out=h[:, t, :], in0=ph[:, 0:D], scalar=1.0, in1=bin_t[:],
                    op0=OP.mult, op1=OP.add,
                )
                _lnproj_block(t, 0)

            # ---- layers ----------------------------------------------------
            for layer in range(L):
                kvf = kvfs[layer % 2]
                # ---- edge phase: per (tgt block, bucket) run of trun tiles
                for blk in range(NBLK):
                    pm = psC.tile([P, D + H], dt.float32, tag="pm")
                    for b in range(2):
                        off = (blk * 2 + b) * EDB
                        kit = gp.tile([P, KIB], dt.uint8, tag="kit")
                        nc.sync.dma_start(kit[:], ed_p.ap()[:, off : off + KIB])
                        ed = gp.tile([P, 2 * nrun], dt.uint8, tag="ed")
                        nc.sync.dma_start(ed[:], ed_p.ap()[:, off + KIB : off + EDB])
                        ki = kit[:].bitcast(dt.int16)               # [P, trun*8]
                        sgv = ed[:, 0:nrun].bitcast(dt.float8e4).rearrange(
                            "p (t e) -> p t e", e=P
                        )
                        stv = ed[:, nrun : 2 * nrun].bitcast(dt.float8e4).rearrange(
                            "p (t e) -> p t e", e=P
                        )

                        if ABL_NOEDGE:
                            ue0 = ep.tile([P, trun, D + H], dt.bfloat16, tag="ue")
                            nc.vector.memset(ue0[:], 0.5)
                            for tt in range(trun):
                                nc.tensor.matmul(
                                    out=pm[:], lhsT=sgv[:, tt, :], rhs=ue0[:, tt, :],
                                    start=(b == 0 and tt == 0),
                                    stop=(b == 1 and tt == trun - 1),
                                )
                            continue
                        kvg = gp.tile([P, trun, 2 * D], dt.bfloat16, tag="kvg")
                        for ci, i0 in enumerate(range(0, trun, GBATCH)):
                            i1 = min(i0 + GBATCH, trun)
                            nidx = (i1 - i0) * P
                            if min_run >= i1 * P:
                                creg = nidx  # window always full: static count
                            else:
                                gj = (blk * 2 + b) * NC_G + ci
                                creg = gregs[gregc[0] % len(gregs)]
                                gregc[0] += 1
                                nc.gpsimd.reg_load(creg, gct[0:1, gj : gj + 1])
                            nc.gpsimd.dma_gather(
                                out_ap=kvg[:, i0:i1, :],
                                in_ap=kvf.ap()[b * NBUK0 : b * NBUK0 + (NBUK1 if b else NBUK0), :],
                                idxs_ap=ki[:, i0 * 8 : i1 * 8],
                                num_idxs=nidx, num_idxs_reg=creg,
                                elem_size=2 * D,
                                queue_num=_nextq(),
                            )
                        # q-broadcast via PE: qg[e, f] = q[tl(e), f]
                        qg = ep.tile([P, trun, D], dt.bfloat16, tag="qg")
                        for c0 in range(0, nrun, 512):
                            c1 = min(c0 + 512, nrun)
                            qp = psQ.tile([P, 512], dt.float32, tag="qp")
                            for tt in range(c0 // P, c1 // P):
                                o = tt * P - c0
                                nc.tensor.matmul(
                                    out=qp[:, o : o + P], lhsT=stv[:, tt, :],
                                    rhs=qsb[:, blk, :], start=True, stop=True,
                                )
                            nc.scalar.copy(
                                qg[:, c0 // P : c1 // P, :],
                                qp[:, 0 : c1 - c0].rearrange("p (t e) -> p t e", e=P),
                            )
                        # per-edge logits: alpha = sum_c q*k (tree reduce)
                        qk = ep.tile([P, trun, H, C], dt.bfloat16, tag="qk")
                        nc.vector.tensor_tensor(
                            out=qk[:].rearrange("p t h c -> p t (h c)"),
                            in0=qg[:], in1=kvg[:, :, 0:D], op=OP.mult,
                        )
                        t1 = ep.tile([P, trun, H, 4], dt.bfloat16, tag="t1")
                        with nc.allow_low_precision(reason="alpha logits are O(0.1)"):
                            nc.vector.tensor_tensor(
                                out=t1[:], in0=qk[:, :, :, 0:4], in1=qk[:, :, :, 4:8],
                                op=OP.add,
                            )
                            t2 = ep.tile([P, trun, H, 2], dt.bfloat16, tag="t2")
                            nc.vector.tensor_tensor(
                                out=t2[:], in0=t1[:, :, :, 0:2], in1=t1[:, :, :, 2:4],
                                op=OP.add,
                            )
                            al = ep.tile([P, trun, H, 1], dt.bfloat16, tag="al")
                            nc.vector.tensor_tensor(
                                out=al[:], in0=t2[:, :, :, 0:1], in1=t2[:, :, :, 1:2],
                                op=OP.add,
                            )
                        ue = ep.tile([P, trun, D + H], dt.bfloat16, tag="ue")
                        nc.scalar.activation(
                            out=ue[:, :, D : D + H].rearrange("p t (h o) -> p t h o", o=1),
                            in_=al[:], func=AF.Exp,
                        )
                        wex = ep.tile([P, trun, H, C], dt.bfloat16, tag="wex")
                        nc.scalar.activation(
                            out=wex[:], in_=al[:].to_broadcast([P, trun, H, C]),
                            func=AF.Exp,
                        )
                        nc.vector.tensor_tensor(
                            out=ue[:, :, 0:D], in0=kvg[:, :, D : 2 * D],
                            in1=wex[:].rearrange("p t h c -> p t (h c)"), op=OP.mult,
                        )
                        for tt in range(trun):
                            nc.tensor.matmul(
                                out=pm[:], lhsT=sgv[:, tt, :], rhs=ue[:, tt, :],
                                start=(b == 0 and tt == 0),
                                stop=(b == 1 and tt == trun - 1),
                            )

                    # ---- fused per-block tail: normalize, gate, residual,
                    # then next layer's LN+projection (or final LN) ----------
                    msgb = sp.tile([P, D], dt.float32, tag="msgb")
                    rden = sp.tile([P, H, 1], dt.float32, tag="rden")
                    nc.vector.tensor_scalar_add(
                        rden[:], pm[:, D : D + H].rearrange("p (h o) -> p h o", o=1), 1e-20
                    )
                    nc.vector.reciprocal(rden[:], rden[:])
                    nc.vector.tensor_tensor(
                        out=msgb[:].rearrange("p (h c) -> p h c", c=C),
                        in0=pm[:, 0:D].rearrange("p (h c) -> p h c", c=C),
                        in1=rden[:].to_broadcast([P, H, C]),
                        op=OP.mult,
                    )
                    nc.vector.tensor_tensor(
                        out=msgb[:], in0=msgb[:], in1=bvt[:, layer, :], op=OP.add
                    )
                    scr = sp.tile([P, D], dt.float32, tag="scr")
                    bs1 = sp.tile([P, 1], dt.float32, tag="bs1")
                    nc.vector.scalar_tensor_tensor(
                        out=scr[:], in0=msgb[:], scalar=1.0, in1=wb[:, layer, 0:D],
                        op0=OP.mult, op1=OP.mult, accum_out=bs1[:],
                    )
                    bs2 = sp.tile([P, 1], dt.float32, tag="bs2")
                    nc.vector.scalar_tensor_tensor(
                        out=scr[:], in0=xr[:, blk, :], scalar=1.0, in1=wb[:, layer, D : 2 * D],
                        op0=OP.mult, op1=OP.mult, accum_out=bs2[:],
                    )
                    nc.vector.tensor_tensor(out=bs1[:], in0=bs1[:], in1=bs2[:], op=OP.add)
                    beta = sp.tile([P, 1], dt.float32, tag="beta")
                    nc.scalar.activation(out=beta[:], in_=bs1[:], func=AF.Exp, scale=-1.0)
                    nc.vector.tensor_scalar_add(beta[:], beta[:], 1.0)
                    nc.vector.reciprocal(beta[:], beta[:])
                    # h += msg + beta*(xr - msg)
                    tmpb = sp.tile([P, D], dt.float32, tag="tmpb")
                    nc.vector.tensor_tensor(
                        out=tmpb[:], in0=xr[:, blk, :], in1=msgb[:], op=OP.subtract
                    )
                    nc.vector.scalar_tensor_tensor(
                        out=tmpb[:], in0=tmpb[:], scalar=beta[:], in1=msgb[:],
                        op0=OP.mult, op1=OP.add,
                    )
                    nc.vector.tensor_tensor(
                        out=h[:, blk, :], in0=h[:, blk, :], in1=tmpb[:], op=OP.add
                    )
                    if layer == L - 1:
                        _final_block(blk)
                    else:
                        _lnproj_block(blk, layer + 1)

    nc.finalize()
    return nc

LAST_RES = None


def _make_in_maps(inputs, cores):
    sq = 1.0 / np.sqrt(np.float32(C))
    Wq, Wk, Wv, Wsk = (np.asarray(inputs[k], dtype=np.float32) for k in ("Wq", "Wk", "Wv", "Wskip"))
    bq, bv, bsk = (np.asarray(inputs[k], dtype=np.float32) for k in ("bq", "bv", "bskip"))
    # order per layer: k | v | q*sq | skip  (k-bias dropped: softmax shift
    # invariance; v-bias folded in post-aggregation)
    wcat = np.concatenate([Wk, Wv, Wq * sq, Wsk], axis=2).transpose(1, 0, 2).reshape(D, L * 4 * D)
    bcat = np.concatenate([bq * sq, bsk], axis=1)  # [L, 2D]
    bcat_rep = np.broadcast_to(bcat[:, None, :], (L, P, 2 * D)).transpose(1, 0, 2).reshape(P, L * 2 * D).copy()
    bvp = np.broadcast_to(bv[:, None, :], (L, P, D)).transpose(1, 0, 2).reshape(P, L * D).copy()
    lns, lnb = np.asarray(inputs["ln_scale"], np.float32), np.asarray(inputs["ln_bias"], np.float32)
    lnp = np.broadcast_to(
        np.concatenate([lns, lnb], axis=1)[:, None, :], (L, P, 2 * D)
    ).transpose(1, 0, 2).reshape(P, L * 2 * D).copy()
    fnp = np.broadcast_to(
        np.concatenate([inputs["fn_scale"], inputs["fn_bias"]])[None, :], (P, 2 * D)
    ).astype(np.float32).copy()
    Wbeta = np.asarray(inputs["Wbeta"], np.float32)  # [L, 3D, 1]
    wa = Wbeta[:, 0:D, 0] + Wbeta[:, 2 * D : 3 * D, 0]      # msg coeff
    wbx = Wbeta[:, D : 2 * D, 0] - Wbeta[:, 2 * D : 3 * D, 0]  # xr coeff
    wbeta_rep = np.broadcast_to(
        np.concatenate([wa, wbx], axis=1)[:, None, :], (L, P, 2 * D)
    ).transpose(1, 0, 2).reshape(P, L * 2 * D).copy()
    bin_rep = np.broadcast_to(
        np.asarray(inputs["b_in"], np.float32)[None, :], (P, D)
    ).copy()

    common = dict(
        win=_bf16(inputs["W_in"]),
        bin=bin_rep,
        wcat=_bf16(wcat),
        bcat=bcat_rep,
        bvp=bvp,
        lnp=lnp,
        fnp=fnp,
        wbeta=wbeta_rep,
    )
    KROWS = cores[0]["degoh"].shape[0]
    embio = np.concatenate(
        [np.asarray(inputs["in_emb"], np.float32)[0:KROWS],
         np.asarray(inputs["out_emb"], np.float32)[0:KROWS]], axis=1
    )
    common["embio"] = _bf16(embio)
    in_maps = []
    for c in range(NCORES):
        m = dict(common)
        cd = cores[c]
        m.update(x=cd["x"], edata=cd["edata"], gcnt=cd["gcnt"], degoh=cd["degoh"])
        in_maps.append(m)
    return in_maps


def kernel(**inputs):
    import os

    from concourse.bass_utils import run_bass_kernel_spmd

    x = np.asarray(inputs["x"], dtype=np.float32)
    edge_index = np.asarray(inputs["edge_index"])
    cores, trun, EDB, min_run, KROWS = _preprocess(x, edge_index)
    in_maps = _make_in_maps(inputs, cores)

    nc = _build(trun, EDB, min_run, KROWS)
    kw = {}
    td = os.environ.get("BASS_KERNEL_TMPDIR")
    if td:
        kw["tmpdir"] = td
    res = run_bass_kernel_spmd(nc, in_maps, core_ids=list(range(NCORES)), **kw)
    global LAST_RES
    LAST_RES = res
    outs = [np.asarray(r["out"], dtype=np.float32) for r in res.results]
    return np.concatenate(outs, axis=0)


if __name__ == "__main__":
    import reference

    inp = {k: np.asarray(v) for k, v in reference.setup_inputs().items()}
    exp = np.asarray(reference.reference(**inp))
    act = kernel(**inp)
    err = np.abs(act - exp).max() / (np.abs(exp).max() + 1e-9)
    print("Relative error:", err)


# revision 35
# speedup vs baseline: 1.0023x; 1.0023x over previous
"""Graphormer-expert GNN kernel for 8 Trainium2 NeuronCores.

Strategy (matches the sharding hint): nodes are partitioned 8 x 6250 (graph
parallel); each core owns the edges whose *target* falls in its shard, so the
scatter-softmax is core-local.  Per layer each core computes LN + projections
for its own nodes, k|v rows (bf16) are exchanged with an AllGather, and
per-edge k/v rows are fetched with SWDGE dma_gather (int16 indices; source
table split in two 25088-row buckets; padding indices are -1 so the Q7
descriptor generator trims them).  Per-edge softmax runs without
max-subtraction (|alpha| << 1 for this model); the q-row broadcast and the
segment sums (softmax denominator + message aggregation) run on the
TensorEngine with host-precomputed fp8 one-hot matrices (exact), accumulating
each target-block's edge tiles in PSUM.  The softmax division is commuted
past the segment sum and applied per node.  k-bias drops out exactly (softmax
shift invariance); v-bias is folded in after aggregation.  ACT runs only
copies + Exp (sigmoid via exp, layernorm rsqrt via DVE Newton iteration) so
the activation table is loaded once.
"""

import sys

sys.path.insert(0, "/opt/trn_rl_repo")

import numpy as np

N, IN_DIM, D, H, L, E, MAX_DEG = 50000, 128, 128, 16, 3, 800000, 512
C = D // H
P = 128
NCORES = 8
NSH = N // NCORES            # 6250 nodes per core
NBLK = (NSH + P - 1) // P    # 49 target blocks per core
NPAD = NBLK * P              # 6272 padded rows per core
CH_BLK = [0, 12, 24, 36, 48, 49]      # AllGather chunk boundaries (blocks)
CH_ROW = [b * P for b in CH_BLK]       # per-core row boundaries
NBUK0 = 8 * CH_ROW[2]                  # bucket 0 = chunks 0-1 = 24576 rows
NBUK1 = 8 * (CH_ROW[-1] - CH_ROW[2])   # bucket 1 = chunks 2.. = 25600 rows

RSQRT_MAGIC = 0x5F3759DF


def _bf16(a):
    import ml_dtypes

    return np.asarray(a, dtype=ml_dtypes.bfloat16)


def _fp8(a):
    import ml_dtypes

    return np.asarray(a, dtype=ml_dtypes.float8_e4m3)


def _wrap_idx16(idx, pad_to=None, fill=0):
    """int16 idx array -> [128, n/16] wrapped (j -> [j%16, j//16]) and
    replicated across the 8 gpsimd cores' 16-partition groups."""
    n = len(idx) if pad_to is None else pad_to
    assert n % 16 == 0
    a = np.full(n, fill, dtype=np.int16)
    a[: len(idx)] = idx.astype(np.int16)
    w = a.reshape(n // 16, 16).T  # [16, n/16]
    return np.tile(w, (8, 1))  # [128, n/16]


def _preprocess(x, edge_index):
    """Host-side integer/index preprocessing + per-core shard arrays."""
    src = np.asarray(edge_index[0], dtype=np.int64)
    tgt = np.asarray(edge_index[1], dtype=np.int64)

    # degrees (int) for the centrality embeddings (applied as one-hot matmuls)
    idg = np.clip(np.bincount(tgt, minlength=N), 0, MAX_DEG)
    odg = np.clip(np.bincount(src, minlength=N), 0, MAX_DEG)
    dmax = int(max(idg.max(), odg.max()))
    KROWS = 64 if dmax < 64 else 128
    assert dmax < KROWS

    # global row in the chunked-AllGather kv table of node g: chunk-major,
    # then core, then row-within-chunk
    sc = src // NSH
    sr = src % NSH
    chv = np.searchsorted(np.asarray(CH_ROW), sr, side="right") - 1
    nch = len(CH_ROW) - 1
    chw = np.asarray([CH_ROW[i + 1] - CH_ROW[i] for i in range(nch)])
    ch_base8 = np.asarray([8 * CH_ROW[i] for i in range(nch)])
    kv_row = ch_base8[chv] + sc * chw[chv] + (sr - np.asarray(CH_ROW)[chv])
    bucket = (kv_row >= NBUK0).astype(np.int64)
    src_loc = kv_row - bucket * NBUK0  # int16-safe (< 25600)

    # first pass: find the max (block,bucket) run length across all cores
    run_max = 0
    per_core = []
    for c in range(NCORES):
        m = (tgt // NSH) == c
        cs, ct, cb, csl = src[m], tgt[m] - c * NSH, bucket[m], src_loc[m]
        blk = ct // P
        cnt = np.bincount(blk * 2 + cb, minlength=NBLK * 2)
        run_max = max(run_max, cnt.max())
        per_core.append((cs, ct, cb, csl, blk))
    trun = int((run_max + P - 1) // P)  # tiles per (block,bucket) run
    nrun = trun * P
    KIB = trun * 16                     # idx bytes per partition per run
    EDB = KIB + 2 * trun * P            # + sg fp8 + st fp8 bytes

    ncalls = (trun + GBATCH - 1) // GBATCH
    min_run = min(
        int(np.bincount(blk * 2 + cb, minlength=NBLK * 2).min())
        for (_, _, cb, _, blk) in per_core
    )
    cores = []
    for c in range(NCORES):
        cs, ct, cb, csl, blk = per_core[c]
        order = np.lexsort((ct, cb, blk))
        cs, ct, cb, csl, blk = (a[order] for a in (cs, ct, cb, csl, blk))

        edata = np.zeros((P, NBLK * 2 * EDB), dtype=np.uint8)
        gcnt = np.zeros((NBLK * 2, ncalls), dtype=np.int32)
        for k in range(NBLK):
            for b in range(2):
                sel = (cb == b) & (blk == k)
                n_e = int(sel.sum())
                for ci, i0 in enumerate(range(0, trun, GBATCH)):
                    i1 = min(i0 + GBATCH, trun)
                    gcnt[k * 2 + b, ci] = max(0, min(n_e - i0 * P, (i1 - i0) * P))
                ki = _wrap_idx16(csl[sel], pad_to=nrun, fill=-1)  # [128, trun*8]
                tl = ct[sel] - k * P  # 0..127 col within the block
                ee = np.arange(n_e)
                S = np.zeros((P, nrun), dtype=np.float32)   # [e%128, t*128+tl]
                S[ee % P, (ee // P) * P + tl] = 1.0
                ST = np.zeros((P, nrun), dtype=np.float32)  # [tl, e]
                ST[tl, ee] = 1.0
                off = (k * 2 + b) * EDB
                edata[:, off : off + KIB] = ki.view(np.uint8)
                edata[:, off + KIB : off + KIB + nrun] = _fp8(S).view(np.uint8)
                edata[:, off + KIB + nrun : off + EDB] = _fp8(ST).view(np.uint8)

        cidg = np.pad(idg[c * NSH : (c + 1) * NSH], (0, NPAD - NSH))
        codg = np.pad(odg[c * NSH : (c + 1) * NSH], (0, NPAD - NSH))
        degoh = np.zeros((KROWS, NBLK * 2 * P), dtype=np.float32)
        for k in range(NBLK):
            nodes = np.arange(k * P, (k + 1) * P)
            degoh[cidg[nodes], k * 2 * P + np.arange(P)] = 1.0
            degoh[codg[nodes], k * 2 * P + P + np.arange(P)] = 1.0

        cores.append(
            dict(
                edata=edata,
                gcnt=np.broadcast_to(gcnt.reshape(1, -1), (P, NBLK * 2 * ncalls)).copy(),
                degoh=_fp8(degoh),
                x=_bf16(np.pad(
                    np.asarray(x[c * NSH : (c + 1) * NSH], dtype=np.float32),
                    ((0, NPAD - NSH), (0, 0)),
                ).T.copy()),
            )
        )
    return cores, trun, EDB, min_run, KROWS


import os as _os

PROBE_NO_COLLECTIVE = bool(int(_os.environ.get("KB_NOCOLL", "0")))
ABL_NOEDGE = bool(int(_os.environ.get("KB_NOEDGE", "0")))   # skip edge interior

GBATCH = int(_os.environ.get("KB_GBATCH", "5"))   # kv gather tiles per call
EBATCH = int(_os.environ.get("KB_EBATCH", "3"))   # emb gather blocks per call
EPBUFS = int(_os.environ.get("KB_EPBUFS", "4"))   # edge pool depth
GPBUFS = int(_os.environ.get("KB_GPBUFS", "7"))   # gather pool depth (ed+kvg)


def _build(trun, EDB, min_run=0, KROWS=64):
    from concourse import bass, mybir
    import concourse.tile as tile
    from concourse.bacc import Bacc
    from concourse.masks import make_identity

    dt = mybir.dt
    AX = mybir.AxisListType
    OP = mybir.AluOpType
    AF = mybir.ActivationFunctionType

    KIB = trun * 16
    nrun = trun * P

    nc = Bacc(None, target_bir_lowering=False, debug=False, num_devices=NCORES,
              num_swdge_queues=4)
    qctr = [0]

    def _nextq():
        qctr[0] = (qctr[0] + 1) % 4
        return qctr[0]

    # ---- parameters (per core) -------------------------------------------
    xin = nc.declare_dram_parameter("x", [D, NPAD], dt.bfloat16, isOutput=False)
    deg_p = nc.declare_dram_parameter("degoh", [KROWS, NBLK * 2 * P], dt.float8e4, isOutput=False)
    eio_p = nc.declare_dram_parameter("embio", [KROWS, 2 * D], dt.bfloat16, isOutput=False)
    win_p = nc.declare_dram_parameter("win", [D, D], dt.bfloat16, isOutput=False)
    bin_p = nc.declare_dram_parameter("bin", [P, D], dt.float32, isOutput=False)
    wcat_p = nc.declare_dram_parameter("wcat", [D, L * 4 * D], dt.bfloat16, isOutput=False)
    bcat_p = nc.declare_dram_parameter("bcat", [P, L * 2 * D], dt.float32, isOutput=False)
    bvp_p = nc.declare_dram_parameter("bvp", [P, L * D], dt.float32, isOutput=False)
    lnp_p = nc.declare_dram_parameter("lnp", [P, L * 2 * D], dt.float32, isOutput=False)
    fnp_p = nc.declare_dram_parameter("fnp", [P, 2 * D], dt.float32, isOutput=False)
    wb_p = nc.declare_dram_parameter("wbeta", [P, L * 2 * D], dt.float32, isOutput=False)
    ed_p = nc.declare_dram_parameter("edata", [P, NBLK * 2 * EDB], dt.uint8, isOutput=False)
    NC_G = (trun + GBATCH - 1) // GBATCH
    gc_p = nc.declare_dram_parameter("gcnt", [P, NBLK * 2 * NC_G], dt.int32, isOutput=False)
    out_p = nc.declare_dram_parameter("out", [NSH, D], dt.float32, isOutput=True)

    # ---- DRAM scratch -----------------------------------------------------
    kvb = nc.dram_tensor("kv_bounce", [NPAD, 2 * D], dt.bfloat16)
    kvfs = [
        nc.dram_tensor(f"kv_full{i}", [NCORES * NPAD, 2 * D], dt.bfloat16, addr_space="Shared")
        for i in range(2)
    ]

    with tile.TileContext(nc) as tc:
        with (
            tc.tile_pool(name="persist", bufs=1) as pp,
            tc.tile_pool(name="wtiles", bufs=1) as wp,
            tc.tile_pool(name="work", bufs=1) as kp,
            tc.tile_pool(name="small", bufs=3) as sp,
            tc.tile_pool(name="edge", bufs=EPBUFS) as ep,
            tc.tile_pool(name="gath", bufs=GPBUFS) as gp,
            tc.tile_pool(name="psA", bufs=1, space="PSUM") as psA,
            tc.tile_pool(name="psB", bufs=2, space="PSUM") as psB,
            tc.tile_pool(name="psC", bufs=2, space="PSUM") as psC,
            tc.tile_pool(name="psQ", bufs=2, space="PSUM") as psQ,
        ):
            # persistent state
            h = pp.tile([P, NBLK, D], dt.float32, tag="h")
            xr = pp.tile([P, NBLK, D], dt.bfloat16, tag="xr")
            qsb = pp.tile([P, NBLK, D], dt.bfloat16, tag="qsb")

            ident = wp.tile([P, P], dt.bfloat16, tag="ident")
            make_identity(nc, ident[:])
            win = wp.tile([D, D], dt.bfloat16, tag="win")
            nc.sync.dma_start(win[:], win_p.ap())
            bin_t = wp.tile([P, D], dt.float32, tag="bin")
            nc.sync.dma_start(bin_t[:], bin_p.ap())
            wcat = wp.tile([D, L, 4 * D], dt.bfloat16, tag="wcat")
            nc.sync.dma_start(wcat[:], wcat_p.ap())
            bcat = wp.tile([P, L, 2 * D], dt.float32, tag="bcat")
            nc.sync.dma_start(bcat[:], bcat_p.ap())
            bvt = wp.tile([P, L, D], dt.float32, tag="bvt")
            nc.sync.dma_start(bvt[:], bvp_p.ap())
            lnp = wp.tile([P, L, 2 * D], dt.float32, tag="lnp")
            nc.sync.dma_start(lnp[:], lnp_p.ap())
            fnp = wp.tile([P, 2 * D], dt.float32, tag="fnp")
            nc.sync.dma_start(fnp[:], fnp_p.ap())
            wb = wp.tile([P, L, 2 * D], dt.float32, tag="wb")
            nc.sync.dma_start(wb[:], wb_p.ap())
            gct = wp.tile([P, NBLK * 2 * NC_G], dt.int32, tag="gct")
            nc.sync.dma_start(gct[:], gc_p.ap())
            xTt = wp.tile([D, NBLK, P], dt.bfloat16, tag="xTt")
            nc.sync.dma_start(xTt[:], xin.ap())
            eio = wp.tile([KROWS, 2 * D], dt.bfloat16, tag="eio")
            nc.sync.dma_start(eio[:], eio_p.ap())
            deg = wp.tile([KROWS, NBLK, 2 * P], dt.float8e4, tag="deg")
            nc.sync.dma_start(deg[:], deg_p.ap())
            gregs = [nc.gpsimd.alloc_register(f"gcnt_reg{i}") for i in range(8)]
            gregc = [0]

            def _rsqrt(rs, ve):
                """rs = 1/sqrt(ve) via bit-hack seed + 2 Newton iterations.
                rs, ve: [P, 1] f32 tiles (DVE only — no ACT table)."""
                iv = sp.tile([P, 1], dt.int32, tag="nw_i")
                nc.vector.tensor_scalar(
                    out=iv[:], in0=ve[:].bitcast(dt.int32), scalar1=1,
                    scalar2=None, op0=OP.logical_shift_right,
                )
                nc.vector.tensor_scalar(
                    out=iv[:], in0=iv[:], scalar1=-1, scalar2=RSQRT_MAGIC,
                    op0=OP.mult, op1=OP.add,
                )
                y = iv[:].bitcast(dt.float32)
                t = sp.tile([P, 1], dt.float32, tag="nw_t")
                cur = y
                for it in range(2):
                    nxt = rs[:] if it == 1 else t[:]
                    nc.vector.tensor_tensor(out=nxt, in0=cur, in1=cur, op=OP.mult)
                    nc.vector.tensor_tensor(out=nxt, in0=nxt, in1=ve[:], op=OP.mult)
                    nc.vector.tensor_scalar(
                        out=nxt, in0=nxt, scalar1=-0.5, scalar2=1.5,
                        op0=OP.mult, op1=OP.add,
                    )
                    nc.vector.tensor_tensor(out=nxt, in0=cur, in1=nxt, op=OP.mult)
                    cur = nxt

            def _ln_to(hb, t, scale_ap, bias_ap, act_sqrt=False):
                """hb[P, D] (bf16) = LN(h[:, t, :]) * scale + bias."""
                stats = sp.tile([P, 6], dt.float32, tag="bst")
                nc.vector.bn_stats(stats[:], h[:, t, :])
                mv = sp.tile([P, 2], dt.float32, tag="mv")
                nc.vector.bn_aggr(mv[:], stats[:])
                ve = sp.tile([P, 1], dt.float32, tag="ve")
                nc.vector.tensor_scalar_add(ve[:], mv[:, 1:2], 1e-5)
                rs = sp.tile([P, 1], dt.float32, tag="rs")
                if act_sqrt:
                    nc.scalar.sqrt(rs[:], ve[:])
                    nc.vector.reciprocal(rs[:], rs[:])
                else:
                    _rsqrt(rs, ve)
                hf = sp.tile([P, D], dt.float32, tag="hf")
                nc.vector.tensor_tensor(
                    out=hf[:], in0=h[:, t, :],
                    in1=mv[:, 0:1].to_broadcast([P, D]), op=OP.subtract,
                )
                nc.vector.scalar_tensor_tensor(
                    out=hb[:], in0=hf[:], scalar=rs[:], in1=scale_ap,
                    op0=OP.mult, op1=OP.mult,
                )
                nc.vector.tensor_tensor(out=hb[:], in0=hb[:], in1=bias_ap, op=OP.add)

            def _lnproj_block(t, layer):
                """LN h[:,t] (lnp[layer]) -> proj (wcat[layer]) -> kvb/qsb/xr."""
                hb = sp.tile([P, D], dt.bfloat16, tag="hb")
                _ln_to(hb, t, lnp[:, layer, 0:D], lnp[:, layer, D : 2 * D],
                       act_sqrt=(layer == 0))
                pT = psA.tile([P, P], dt.bfloat16, tag="pT")
                nc.tensor.transpose(out=pT[:], in_=hb[:], identity=ident[:])
                hnTt = sp.tile([P, D], dt.bfloat16, tag="hnTt")
                nc.scalar.copy(hnTt[:], pT[:])
                ps = psB.tile([P, 4 * D], dt.float32, tag="ps")
                nc.tensor.matmul(
                    out=ps[:], lhsT=hnTt[:], rhs=wcat[:, layer, :],
                    start=True, stop=True,
                )
                kvq = sp.tile([P, 2 * D], dt.bfloat16, tag="kvq")
                nc.scalar.copy(kvq[:], ps[:, 0 : 2 * D])
                nc.vector.scalar_tensor_tensor(
                    out=qsb[:, t, :], in0=ps[:, 2 * D : 3 * D], scalar=1.0,
                    in1=bcat[:, layer, 0:D], op0=OP.mult, op1=OP.add,
                )
                nc.vector.scalar_tensor_tensor(
                    out=xr[:, t, :], in0=ps[:, 3 * D : 4 * D], scalar=1.0,
                    in1=bcat[:, layer, D : 2 * D], op0=OP.mult, op1=OP.add,
                )
                nc.scalar.dma_start(kvb.ap()[t * P : (t + 1) * P, :], kvq[:])
                if t + 1 in CH_BLK:
                    ch = CH_BLK.index(t + 1) - 1
                    r0, r1 = CH_ROW[ch], CH_ROW[ch + 1]
                    kvf_l = kvfs[layer % 2]
                    if PROBE_NO_COLLECTIVE:
                        nc.gpsimd.dma_start(
                            out=kvf_l.ap()[8 * r0 : 8 * r0 + (r1 - r0), :],
                            in_=kvb.ap()[r0:r1, :],
                        )
                    else:
                        nc.gpsimd.collective_compute(
                            "AllGather",
                            OP.bypass,
                            replica_groups=[list(range(NCORES))],
                            ins=[kvb.ap()[r0:r1, :].opt()],
                            outs=[kvf_l.ap()[8 * r0 : 8 * r1, :].opt()],
                        )

            def _final_block(t):
                """Final LN on h[:,t] -> out DMA."""
                ot = sp.tile([P, D], dt.float32, tag="ot")
                stats = sp.tile([P, 6], dt.float32, tag="bst")
                nc.vector.bn_stats(stats[:], h[:, t, :])
                mv = sp.tile([P, 2], dt.float32, tag="mv")
                nc.vector.bn_aggr(mv[:], stats[:])
                ve = sp.tile([P, 1], dt.float32, tag="ve")
                nc.vector.tensor_scalar_add(ve[:], mv[:, 1:2], 1e-5)
                rs = sp.tile([P, 1], dt.float32, tag="rs")
                _rsqrt(rs, ve)
                nc.vector.tensor_tensor(
                    out=ot[:], in0=h[:, t, :],
                    in1=mv[:, 0:1].to_broadcast([P, D]), op=OP.subtract,
                )
                nc.vector.scalar_tensor_tensor(
                    out=ot[:], in0=ot[:], scalar=rs[:], in1=fnp[:, 0:D],
                    op0=OP.mult, op1=OP.mult,
                )
                nc.vector.tensor_tensor(
                    out=ot[:], in0=ot[:], in1=fnp[:, D : 2 * D], op=OP.add
                )
                lo = t * P
                hi = min((t + 1) * P, NSH)
                if hi > lo:
                    nc.scalar.dma_start(out_p.ap()[lo:hi, :], ot[0 : hi - lo, :])

            # zero the kvg pool buffers once (trimmed gathers leave stale
            # bytes behind; first use must not see NaN bit patterns)
            for _ in range(GPBUFS):
                z = gp.tile([P, trun, 2 * D], dt.bfloat16, tag="kvg")
                nc.vector.memset(z[:], 0.0)

            # ---- phase 0 (fused with layer-0 LN+proj):
            # h = x @ W_in + b_in + emb_in[idg] + emb_out[odg], the embedding
            # gathers expressed as one-hot matmuls accumulated in PSUM
            for t in range(NBLK):
                ph = psB.tile([P, 4 * D], dt.float32, tag="ps")
                nc.tensor.matmul(out=ph[:, 0:D], lhsT=xTt[:, t, :], rhs=win[:], start=True, stop=False)
                nc.tensor.matmul(
                    out=ph[:, 0:D], lhsT=deg[:, t, 0:P], rhs=eio[:, 0:D],
                    start=False, stop=False,
                )
                nc.tensor.matmul(
                    out=ph[:, 0:D], lhsT=deg[:, t, P : 2 * P], rhs=eio[:, D : 2 * D],
                    start=False, stop=True,
                )
                nc.vector.scalar_tensor_tensor(
                    out=h[:, t, :], in0=ph[:, 0:D], scalar=1.0, in1=bin_t[:],
                    op0=OP.mult, op1=OP.add,
                )
                _lnproj_block(t, 0)

            # ---- layers ----------------------------------------------------
            for layer in range(L):
                kvf = kvfs[layer % 2]
                # ---- edge phase: per (tgt block, bucket) run of trun tiles
                for blk in range(NBLK):
                    pm = psC.tile([P, D + H], dt.float32, tag="pm")
                    for b in range(2):
                        off = (blk * 2 + b) * EDB
                        kit = gp.tile([P, KIB], dt.uint8, tag="kit")
                        nc.sync.dma_start(kit[:], ed_p.ap()[:, off : off + KIB])
                        ed = gp.tile([P, 2 * nrun], dt.uint8, tag="ed")
                        nc.sync.dma_start(ed[:], ed_p.ap()[:, off + KIB : off + EDB])
                        ki = kit[:].bitcast(dt.int16)               # [P, trun*8]
                        sgv = ed[:, 0:nrun].bitcast(dt.float8e4).rearrange(
                            "p (t e) -> p t e", e=P
                        )
                        stv = ed[:, nrun : 2 * nrun].bitcast(dt.float8e4).rearrange(
                            "p (t e) -> p t e", e=P
                        )

                        if ABL_NOEDGE:
                            ue0 = ep.tile([P, trun, D + H], dt.bfloat16, tag="ue")
                            nc.vector.memset(ue0[:], 0.5)
                            for tt in range(trun):
                                nc.tensor.matmul(
                                    out=pm[:], lhsT=sgv[:, tt, :], rhs=ue0[:, tt, :],
                                    start=(b == 0 and tt == 0),
                                    stop=(b == 1 and tt == trun - 1),
                                )
                            continue
                        kvg = gp.tile([P, trun, 2 * D], dt.bfloat16, tag="kvg")
                        for ci, i0 in enumerate(range(0, trun, GBATCH)):
                            i1 = min(i0 + GBATCH, trun)
                            nidx = (i1 - i0) * P
                            if min_run >= i1 * P:
                                creg = nidx  # window always full: static count
                            else:
                                gj = (blk * 2 + b) * NC_G + ci
                                creg = gregs[gregc[0] % len(gregs)]
                                gregc[0] += 1
                                nc.gpsimd.reg_load(creg, gct[0:1, gj : gj + 1])
                            nc.gpsimd.dma_gather(
                                out_ap=kvg[:, i0:i1, :],
                                in_ap=kvf.ap()[b * NBUK0 : b * NBUK0 + (NBUK1 if b else NBUK0), :],
                                idxs_ap=ki[:, i0 * 8 : i1 * 8],
                                num_idxs=nidx, num_idxs_reg=creg,
                                elem_size=2 * D,
                                queue_num=_nextq(),
                            )
                        # q-broadcast via PE: qg[e, f] = q[tl(e), f]
                        qg = ep.tile([P, trun, D], dt.bfloat16, tag="qg")
                        for c0 in range(0, nrun, 512):
                            c1 = min(c0 + 512, nrun)
                            qp = psQ.tile([P, 512], dt.float32, tag="qp")
                            for tt in range(c0 // P, c1 // P):
                                o = tt * P - c0
                                nc.tensor.matmul(
                                    out=qp[:, o : o + P], lhsT=stv[:, tt, :],
                                    rhs=qsb[:, blk, :], start=True, stop=True,
                                )
                            nc.scalar.copy(
                                qg[:, c0 // P : c1 // P, :],
                                qp[:, 0 : c1 - c0].rearrange("p (t e) -> p t e", e=P),
                            )
                        # per-edge logits: alpha = sum_c q*k (tree reduce)
                        qk = ep.tile([P, trun, H, C], dt.bfloat16, tag="qk")
                        nc.vector.tensor_tensor(
                            out=qk[:].rearrange("p t h c -> p t (h c)"),
                            in0=qg[:], in1=kvg[:, :, 0:D], op=OP.mult,
                        )
                        t1 = ep.tile([P, trun, H, 4], dt.bfloat16, tag="t1")
                        with nc.allow_low_precision(reason="alpha logits are O(0.1)"):
                            nc.vector.tensor_tensor(
                                out=t1[:], in0=qk[:, :, :, 0:4], in1=qk[:, :, :, 4:8],
                                op=OP.add,
                            )
                            t2 = ep.tile([P, trun, H, 2], dt.bfloat16, tag="t2")
                            nc.vector.tensor_tensor(
                                out=t2[:], in0=t1[:, :, :, 0:2], in1=t1[:, :, :, 2:4],
                                op=OP.add,
                            )
                            al = ep.tile([P, trun, H, 1], dt.bfloat16, tag="al")
                            nc.vector.tensor_tensor(
                                out=al[:], in0=t2[:, :, :, 0:1], in1=t2[:, :, :, 1:2],
                                op=OP.add,
                            )
                        ue = ep.tile([P, trun, D + H], dt.bfloat16, tag="ue")
                        nc.scalar.activation(
                            out=ue[:, :, D : D + H].rearrange("p t (h o) -> p t h o", o=1),
                            in_=al[:], func=AF.Exp,
                        )
                        wex = ep.tile([P, trun, H, C], dt.bfloat16, tag="wex")
                        nc.scalar.activation(
                            out=wex[:], in_=al[:].to_broadcast([P, trun, H, C]),
                            func=AF.Exp,
                        )
                        nc.vector.tensor_tensor(
                            out=ue[:, :, 0:D], in0=kvg[:, :, D : 2 * D],
                            in1=wex[:].rearrange("p t h c -> p t (h c)"), op=OP.mult,
                        )
                        for tt in range(trun):
                            nc.tensor.matmul(
                                out=pm[:], lhsT=sgv[:, tt, :], rhs=ue[:, tt, :],
                                start=(b == 0 and tt == 0),
                                stop=(b == 1 and tt == trun - 1),
                            )

                    # ---- fused per-block tail: normalize, gate, residual,
                    # then next layer's LN+projection (or final LN) ----------
                    msgb = sp.tile([P, D], dt.float32, tag="msgb")
                    rden = sp.tile([P, H, 1], dt.float32, tag="rden")
                    nc.vector.tensor_scalar_add(
                        rden[:], pm[:, D : D + H].rearrange("p (h o) -> p h o", o=1), 1e-20
                    )
                    nc.vector.reciprocal(rden[:], rden[:])
                    nc.vector.tensor_tensor(
                        out=msgb[:].rearrange("p (h c) -> p h c", c=C),
                        in0=pm[:, 0:D].rearrange("p (h c) -> p h c", c=C),
                        in1=rden[:].to_broadcast([P, H, C]),
                        op=OP.mult,
                    )
                    nc.vector.tensor_tensor(
                        out=msgb[:], in0=msgb[:], in1=bvt[:, layer, :], op=OP.add
                    )
                    scr = sp.tile([P, D], dt.float32, tag="scr")
                    bs1 = sp.tile([P, 1], dt.float32, tag="bs1")
                    nc.vector.scalar_tensor_tensor(
                        out=scr[:], in0=msgb[:], scalar=1.0, in1=wb[:, layer, 0:D],
                        op0=OP.mult, op1=OP.mult, accum_out=bs1[:],
                    )
                    bs2 = sp.tile([P, 1], dt.float32, tag="bs2")
                    nc.vector.scalar_tensor_tensor(
                        out=scr[:], in0=xr[:, blk, :], scalar=1.0, in1=wb[:, layer, D : 2 * D],
                        op0=OP.mult, op1=OP.mult, accum_out=bs2[:],
                    )
                    nc.vector.tensor_tensor(out=bs1[:], in0=bs1[:], in1=bs2[:], op=OP.add)
                    beta = sp.tile([P, 1], dt.float32, tag="beta")
                    nc.scalar.activation(out=beta[:], in_=bs1[:], func=AF.Exp, scale=-1.0)
                    nc.vector.tensor_scalar_add(beta[:], beta[:], 1.0)
                    nc.vector.reciprocal(beta[:], beta[:])
                    # h += msg + beta*(xr - msg)
                    tmpb = sp.tile([P, D], dt.float32, tag="tmpb")
                    nc.vector.tensor_tensor(
                        out=tmpb[:], in0=xr[:, blk, :], in1=msgb[:], op=OP.subtract
                    )
                    nc.vector.scalar_tensor_tensor(
                        out=tmpb[:], in0=tmpb[:], scalar=beta[:], in1=msgb[:],
                        op0=OP.mult, op1=OP.add,
                    )
                    nc.vector.tensor_tensor(
                        out=h[:, blk, :], in0=h[:, blk, :], in1=tmpb[:], op=OP.add
                    )
                    if layer == L - 1:
                        _final_block(blk)
                    else:
                        _lnproj_block(blk, layer + 1)

    nc.finalize()
    return nc

LAST_RES = None


def _make_in_maps(inputs, cores):
    sq = 1.0 / np.sqrt(np.float32(C))
    Wq, Wk, Wv, Wsk = (np.asarray(inputs[k], dtype=np.float32) for k in ("Wq", "Wk", "Wv", "Wskip"))
    bq, bv, bsk = (np.asarray(inputs[k], dtype=np.float32) for k in ("bq", "bv", "bskip"))
    # order per layer: k | v | q*sq | skip  (k-bias dropped: softmax shift
    # invariance; v-bias folded in post-aggregation)
    wcat = np.concatenate([Wk, Wv, Wq * sq, Wsk], axis=2).transpose(1, 0, 2).reshape(D, L * 4 * D)
    bcat = np.concatenate([bq * sq, bsk], axis=1)  # [L, 2D]
    bcat_rep = np.broadcast_to(bcat[:, None, :], (L, P, 2 * D)).transpose(1, 0, 2).reshape(P, L * 2 * D).copy()
    bvp = np.broadcast_to(bv[:, None, :], (L, P, D)).transpose(1, 0, 2).reshape(P, L * D).copy()
    lns, lnb = np.asarray(inputs["ln_scale"], np.float32), np.asarray(inputs["ln_bias"], np.float32)
    lnp = np.broadcast_to(
        np.concatenate([lns, lnb], axis=1)[:, None, :], (L, P, 2 * D)
    ).transpose(1, 0, 2).reshape(P, L * 2 * D).copy()
    fnp = np.broadcast_to(
        np.concatenate([inputs["fn_scale"], inputs["fn_bias"]])[None, :], (P, 2 * D)
    ).astype(np.float32).copy()
    Wbeta = np.asarray(inputs["Wbeta"], np.float32)  # [L, 3D, 1]
    wa = Wbeta[:, 0:D, 0] + Wbeta[:, 2 * D : 3 * D, 0]      # msg coeff
    wbx = Wbeta[:, D : 2 * D, 0] - Wbeta[:, 2 * D : 3 * D, 0]  # xr coeff
    wbeta_rep = np.broadcast_to(
        np.concatenate([wa, wbx], axis=1)[:, None, :], (L, P, 2 * D)
    ).transpose(1, 0, 2).reshape(P, L * 2 * D).copy()
    bin_rep = np.broadcast_to(
        np.asarray(inputs["b_in"], np.float32)[None, :], (P, D)
    ).copy()

    common = dict(
        win=_bf16(inputs["W_in"]),
        bin=bin_rep,
        wcat=_bf16(wcat),
        bcat=bcat_rep,
        bvp=bvp,
        lnp=lnp,
        fnp=fnp,
        wbeta=wbeta_rep,
    )
    KROWS = cores[0]["degoh"].shape[0]
    embio = np.concatenate(
        [np.asarray(inputs["in_emb"], np.float32)[0:KROWS],
         np.asarray(inputs["out_emb"], np.float32)[0:KROWS]], axis=1
    )
    common["embio"] = _bf16(embio)
    in_maps = []
    for c in range(NCORES):
        m = dict(common)
        cd = cores[c]
        m.update(x=cd["x"], edata=cd["edata"], gcnt=cd["gcnt"], degoh=cd["degoh"])
        in_maps.append(m)
    return in_maps


def kernel(**inputs):
    import os

    from concourse.bass_utils import run_bass_kernel_spmd

    x = np.asarray(inputs["x"], dtype=np.float32)
    edge_index = np.asarray(inputs["edge_index"])
    cores, trun, EDB, min_run, KROWS = _preprocess(x, edge_index)
    in_maps = _make_in_maps(inputs, cores)

    nc = _build(trun, EDB, min_run, KROWS)
    kw = {}
    td = os.environ.get("BASS_KERNEL_TMPDIR")
    if td:
        kw["tmpdir"] = td
    res = run_bass_kernel_spmd(nc, in_maps, core_ids=list(range(NCORES)), **kw)
    global LAST_RES
    LAST_RES = res
    outs = [np.asarray(r["out"], dtype=np.float32) for r in res.results]
    return np.concatenate(outs, axis=0)


if __name__ == "__main__":
    import reference

    inp = {k: np.asarray(v) for k, v in reference.setup_inputs().items()}
    exp = np.asarray(reference.reference(**inp))
    act = kernel(**inp)
    err = np.abs(act - exp).max() / (np.abs(exp).max() + 1e-9)
    print("Relative error:", err)


# revision 36
# speedup vs baseline: 1.0041x; 1.0018x over previous
"""Graphormer-expert GNN kernel for 8 Trainium2 NeuronCores.

Strategy (matches the sharding hint): nodes are partitioned 8 x 6250 (graph
parallel); each core owns the edges whose *target* falls in its shard, so the
scatter-softmax is core-local.  Per layer each core computes LN + projections
for its own nodes, k|v rows (bf16) are exchanged with an AllGather, and
per-edge k/v rows are fetched with SWDGE dma_gather (int16 indices; source
table split in two 25088-row buckets; padding indices are -1 so the Q7
descriptor generator trims them).  Per-edge softmax runs without
max-subtraction (|alpha| << 1 for this model); the q-row broadcast and the
segment sums (softmax denominator + message aggregation) run on the
TensorEngine with host-precomputed fp8 one-hot matrices (exact), accumulating
each target-block's edge tiles in PSUM.  The softmax division is commuted
past the segment sum and applied per node.  k-bias drops out exactly (softmax
shift invariance); v-bias is folded in after aggregation.  ACT runs only
copies + Exp (sigmoid via exp, layernorm rsqrt via DVE Newton iteration) so
the activation table is loaded once.
"""

import sys

sys.path.insert(0, "/opt/trn_rl_repo")

import numpy as np

N, IN_DIM, D, H, L, E, MAX_DEG = 50000, 128, 128, 16, 3, 800000, 512
C = D // H
P = 128
NCORES = 8
NSH = N // NCORES            # 6250 nodes per core
NBLK = (NSH + P - 1) // P    # 49 target blocks per core
NPAD = NBLK * P              # 6272 padded rows per core
CH_BLK = [0, 12, 24, 36, 48, 49]      # AllGather chunk boundaries (blocks)
CH_ROW = [b * P for b in CH_BLK]       # per-core row boundaries
NBUK0 = 8 * CH_ROW[2]                  # bucket 0 = chunks 0-1 = 24576 rows
NBUK1 = 8 * (CH_ROW[-1] - CH_ROW[2])   # bucket 1 = chunks 2.. = 25600 rows

RSQRT_MAGIC = 0x5F3759DF


def _bf16(a):
    import ml_dtypes

    return np.asarray(a, dtype=ml_dtypes.bfloat16)


def _fp8(a):
    import ml_dtypes

    return np.asarray(a, dtype=ml_dtypes.float8_e4m3)


def _wrap_idx16(idx, pad_to=None, fill=0):
    """int16 idx array -> [128, n/16] wrapped (j -> [j%16, j//16]) and
    replicated across the 8 gpsimd cores' 16-partition groups."""
    n = len(idx) if pad_to is None else pad_to
    assert n % 16 == 0
    a = np.full(n, fill, dtype=np.int16)
    a[: len(idx)] = idx.astype(np.int16)
    w = a.reshape(n // 16, 16).T  # [16, n/16]
    return np.tile(w, (8, 1))  # [128, n/16]


def _preprocess(x, edge_index):
    """Host-side integer/index preprocessing + per-core shard arrays."""
    src = np.asarray(edge_index[0], dtype=np.int64)
    tgt = np.asarray(edge_index[1], dtype=np.int64)

    # degrees (int) for the centrality embeddings (applied as one-hot matmuls)
    idg = np.clip(np.bincount(tgt, minlength=N), 0, MAX_DEG)
    odg = np.clip(np.bincount(src, minlength=N), 0, MAX_DEG)
    dmax = int(max(idg.max(), odg.max()))
    KROWS = 64 if dmax < 64 else 128
    assert dmax < KROWS

    # global row in the chunked-AllGather kv table of node g: chunk-major,
    # then core, then row-within-chunk
    sc = src // NSH
    sr = src % NSH
    chv = np.searchsorted(np.asarray(CH_ROW), sr, side="right") - 1
    nch = len(CH_ROW) - 1
    chw = np.asarray([CH_ROW[i + 1] - CH_ROW[i] for i in range(nch)])
    ch_base8 = np.asarray([8 * CH_ROW[i] for i in range(nch)])
    kv_row = ch_base8[chv] + sc * chw[chv] + (sr - np.asarray(CH_ROW)[chv])
    bucket = (kv_row >= NBUK0).astype(np.int64)
    src_loc = kv_row - bucket * NBUK0  # int16-safe (< 25600)

    # first pass: find the max (block,bucket) run length across all cores
    run_max = 0
    per_core = []
    for c in range(NCORES):
        m = (tgt // NSH) == c
        cs, ct, cb, csl = src[m], tgt[m] - c * NSH, bucket[m], src_loc[m]
        blk = ct // P
        cnt = np.bincount(blk * 2 + cb, minlength=NBLK * 2)
        run_max = max(run_max, cnt.max())
        per_core.append((cs, ct, cb, csl, blk))
    trun = int((run_max + P - 1) // P)  # tiles per (block,bucket) run
    nrun = trun * P
    KIB = trun * 16                     # idx bytes per partition per run
    EDB = KIB + 2 * trun * P            # + sg fp8 + st fp8 bytes

    ncalls = (trun + GBATCH - 1) // GBATCH
    min_run = min(
        int(np.bincount(blk * 2 + cb, minlength=NBLK * 2).min())
        for (_, _, cb, _, blk) in per_core
    )
    cores = []
    for c in range(NCORES):
        cs, ct, cb, csl, blk = per_core[c]
        order = np.lexsort((ct, cb, blk))
        cs, ct, cb, csl, blk = (a[order] for a in (cs, ct, cb, csl, blk))

        edata = np.zeros((P, NBLK * 2 * EDB), dtype=np.uint8)
        gcnt = np.zeros((NBLK * 2, ncalls), dtype=np.int32)
        for k in range(NBLK):
            for b in range(2):
                sel = (cb == b) & (blk == k)
                n_e = int(sel.sum())
                for ci, i0 in enumerate(range(0, trun, GBATCH)):
                    i1 = min(i0 + GBATCH, trun)
                    gcnt[k * 2 + b, ci] = max(0, min(n_e - i0 * P, (i1 - i0) * P))
                # edge order within a run is free; sort by source row for
                # HBM locality of the gathered kv reads
                so = np.argsort(csl[sel], kind="stable")
                ki = _wrap_idx16(csl[sel][so], pad_to=nrun, fill=-1)  # [128, trun*8]
                tl = (ct[sel] - k * P)[so]  # 0..127 col within the block
                ee = np.arange(n_e)
                S = np.zeros((P, nrun), dtype=np.float32)   # [e%128, t*128+tl]
                S[ee % P, (ee // P) * P + tl] = 1.0
                ST = np.zeros((P, nrun), dtype=np.float32)  # [tl, e]
                ST[tl, ee] = 1.0
                off = (k * 2 + b) * EDB
                edata[:, off : off + KIB] = ki.view(np.uint8)
                edata[:, off + KIB : off + KIB + nrun] = _fp8(S).view(np.uint8)
                edata[:, off + KIB + nrun : off + EDB] = _fp8(ST).view(np.uint8)

        cidg = np.pad(idg[c * NSH : (c + 1) * NSH], (0, NPAD - NSH))
        codg = np.pad(odg[c * NSH : (c + 1) * NSH], (0, NPAD - NSH))
        degoh = np.zeros((KROWS, NBLK * 2 * P), dtype=np.float32)
        for k in range(NBLK):
            nodes = np.arange(k * P, (k + 1) * P)
            degoh[cidg[nodes], k * 2 * P + np.arange(P)] = 1.0
            degoh[codg[nodes], k * 2 * P + P + np.arange(P)] = 1.0

        cores.append(
            dict(
                edata=edata,
                gcnt=np.broadcast_to(gcnt.reshape(1, -1), (P, NBLK * 2 * ncalls)).copy(),
                degoh=_fp8(degoh),
                x=_bf16(np.pad(
                    np.asarray(x[c * NSH : (c + 1) * NSH], dtype=np.float32),
                    ((0, NPAD - NSH), (0, 0)),
                ).T.copy()),
            )
        )
    return cores, trun, EDB, min_run, KROWS


import os as _os

PROBE_NO_COLLECTIVE = bool(int(_os.environ.get("KB_NOCOLL", "0")))
ABL_NOEDGE = bool(int(_os.environ.get("KB_NOEDGE", "0")))   # skip edge interior

GBATCH = int(_os.environ.get("KB_GBATCH", "5"))   # kv gather tiles per call
EBATCH = int(_os.environ.get("KB_EBATCH", "3"))   # emb gather blocks per call
EPBUFS = int(_os.environ.get("KB_EPBUFS", "4"))   # edge pool depth
GPBUFS = int(_os.environ.get("KB_GPBUFS", "7"))   # gather pool depth (ed+kvg)


def _build(trun, EDB, min_run=0, KROWS=64):
    from concourse import bass, mybir
    import concourse.tile as tile
    from concourse.bacc import Bacc
    from concourse.masks import make_identity

    dt = mybir.dt
    AX = mybir.AxisListType
    OP = mybir.AluOpType
    AF = mybir.ActivationFunctionType

    KIB = trun * 16
    nrun = trun * P

    nc = Bacc(None, target_bir_lowering=False, debug=False, num_devices=NCORES,
              num_swdge_queues=4)
    qctr = [0]

    def _nextq():
        qctr[0] = (qctr[0] + 1) % 4
        return qctr[0]

    # ---- parameters (per core) -------------------------------------------
    xin = nc.declare_dram_parameter("x", [D, NPAD], dt.bfloat16, isOutput=False)
    deg_p = nc.declare_dram_parameter("degoh", [KROWS, NBLK * 2 * P], dt.float8e4, isOutput=False)
    eio_p = nc.declare_dram_parameter("embio", [KROWS, 2 * D], dt.bfloat16, isOutput=False)
    win_p = nc.declare_dram_parameter("win", [D, D], dt.bfloat16, isOutput=False)
    bin_p = nc.declare_dram_parameter("bin", [P, D], dt.float32, isOutput=False)
    wcat_p = nc.declare_dram_parameter("wcat", [D, L * 4 * D], dt.bfloat16, isOutput=False)
    bcat_p = nc.declare_dram_parameter("bcat", [P, L * 2 * D], dt.float32, isOutput=False)
    bvp_p = nc.declare_dram_parameter("bvp", [P, L * D], dt.float32, isOutput=False)
    lnp_p = nc.declare_dram_parameter("lnp", [P, L * 2 * D], dt.float32, isOutput=False)
    fnp_p = nc.declare_dram_parameter("fnp", [P, 2 * D], dt.float32, isOutput=False)
    wb_p = nc.declare_dram_parameter("wbeta", [P, L * 2 * D], dt.float32, isOutput=False)
    ed_p = nc.declare_dram_parameter("edata", [P, NBLK * 2 * EDB], dt.uint8, isOutput=False)
    NC_G = (trun + GBATCH - 1) // GBATCH
    gc_p = nc.declare_dram_parameter("gcnt", [P, NBLK * 2 * NC_G], dt.int32, isOutput=False)
    out_p = nc.declare_dram_parameter("out", [NSH, D], dt.float32, isOutput=True)

    # ---- DRAM scratch -----------------------------------------------------
    kvb = nc.dram_tensor("kv_bounce", [NPAD, 2 * D], dt.bfloat16)
    kvfs = [
        nc.dram_tensor(f"kv_full{i}", [NCORES * NPAD, 2 * D], dt.bfloat16, addr_space="Shared")
        for i in range(2)
    ]

    with tile.TileContext(nc) as tc:
        with (
            tc.tile_pool(name="persist", bufs=1) as pp,
            tc.tile_pool(name="wtiles", bufs=1) as wp,
            tc.tile_pool(name="work", bufs=1) as kp,
            tc.tile_pool(name="small", bufs=3) as sp,
            tc.tile_pool(name="edge", bufs=EPBUFS) as ep,
            tc.tile_pool(name="gath", bufs=GPBUFS) as gp,
            tc.tile_pool(name="psA", bufs=1, space="PSUM") as psA,
            tc.tile_pool(name="psB", bufs=2, space="PSUM") as psB,
            tc.tile_pool(name="psC", bufs=2, space="PSUM") as psC,
            tc.tile_pool(name="psQ", bufs=2, space="PSUM") as psQ,
        ):
            # persistent state
            h = pp.tile([P, NBLK, D], dt.float32, tag="h")
            xr = pp.tile([P, NBLK, D], dt.bfloat16, tag="xr")
            qsb = pp.tile([P, NBLK, D], dt.bfloat16, tag="qsb")

            ident = wp.tile([P, P], dt.bfloat16, tag="ident")
            make_identity(nc, ident[:])
            win = wp.tile([D, D], dt.bfloat16, tag="win")
            nc.sync.dma_start(win[:], win_p.ap())
            bin_t = wp.tile([P, D], dt.float32, tag="bin")
            nc.sync.dma_start(bin_t[:], bin_p.ap())
            wcat = wp.tile([D, L, 4 * D], dt.bfloat16, tag="wcat")
            nc.sync.dma_start(wcat[:], wcat_p.ap())
            bcat = wp.tile([P, L, 2 * D], dt.float32, tag="bcat")
            nc.sync.dma_start(bcat[:], bcat_p.ap())
            bvt = wp.tile([P, L, D], dt.float32, tag="bvt")
            nc.sync.dma_start(bvt[:], bvp_p.ap())
            lnp = wp.tile([P, L, 2 * D], dt.float32, tag="lnp")
            nc.sync.dma_start(lnp[:], lnp_p.ap())
            fnp = wp.tile([P, 2 * D], dt.float32, tag="fnp")
            nc.sync.dma_start(fnp[:], fnp_p.ap())
            wb = wp.tile([P, L, 2 * D], dt.float32, tag="wb")
            nc.sync.dma_start(wb[:], wb_p.ap())
            gct = wp.tile([P, NBLK * 2 * NC_G], dt.int32, tag="gct")
            nc.sync.dma_start(gct[:], gc_p.ap())
            xTt = wp.tile([D, NBLK, P], dt.bfloat16, tag="xTt")
            nc.sync.dma_start(xTt[:], xin.ap())
            eio = wp.tile([KROWS, 2 * D], dt.bfloat16, tag="eio")
            nc.sync.dma_start(eio[:], eio_p.ap())
            deg = wp.tile([KROWS, NBLK, 2 * P], dt.float8e4, tag="deg")
            nc.sync.dma_start(deg[:], deg_p.ap())
            gregs = [nc.gpsimd.alloc_register(f"gcnt_reg{i}") for i in range(8)]
            gregc = [0]

            def _rsqrt(rs, ve):
                """rs = 1/sqrt(ve) via bit-hack seed + 2 Newton iterations.
                rs, ve: [P, 1] f32 tiles (DVE only — no ACT table)."""
                iv = sp.tile([P, 1], dt.int32, tag="nw_i")
                nc.vector.tensor_scalar(
                    out=iv[:], in0=ve[:].bitcast(dt.int32), scalar1=1,
                    scalar2=None, op0=OP.logical_shift_right,
                )
                nc.vector.tensor_scalar(
                    out=iv[:], in0=iv[:], scalar1=-1, scalar2=RSQRT_MAGIC,
                    op0=OP.mult, op1=OP.add,
                )
                y = iv[:].bitcast(dt.float32)
                t = sp.tile([P, 1], dt.float32, tag="nw_t")
                cur = y
                for it in range(2):
                    nxt = rs[:] if it == 1 else t[:]
                    nc.vector.tensor_tensor(out=nxt, in0=cur, in1=cur, op=OP.mult)
                    nc.vector.tensor_tensor(out=nxt, in0=nxt, in1=ve[:], op=OP.mult)
                    nc.vector.tensor_scalar(
                        out=nxt, in0=nxt, scalar1=-0.5, scalar2=1.5,
                        op0=OP.mult, op1=OP.add,
                    )
                    nc.vector.tensor_tensor(out=nxt, in0=cur, in1=nxt, op=OP.mult)
                    cur = nxt

            def _ln_to(hb, t, scale_ap, bias_ap, act_sqrt=False):
                """hb[P, D] (bf16) = LN(h[:, t, :]) * scale + bias."""
                stats = sp.tile([P, 6], dt.float32, tag="bst")
                nc.vector.bn_stats(stats[:], h[:, t, :])
                mv = sp.tile([P, 2], dt.float32, tag="mv")
                nc.vector.bn_aggr(mv[:], stats[:])
                ve = sp.tile([P, 1], dt.float32, tag="ve")
                nc.vector.tensor_scalar_add(ve[:], mv[:, 1:2], 1e-5)
                rs = sp.tile([P, 1], dt.float32, tag="rs")
                if act_sqrt:
                    nc.scalar.sqrt(rs[:], ve[:])
                    nc.vector.reciprocal(rs[:], rs[:])
                else:
                    _rsqrt(rs, ve)
                hf = sp.tile([P, D], dt.float32, tag="hf")
                nc.vector.tensor_tensor(
                    out=hf[:], in0=h[:, t, :],
                    in1=mv[:, 0:1].to_broadcast([P, D]), op=OP.subtract,
                )
                nc.vector.scalar_tensor_tensor(
                    out=hb[:], in0=hf[:], scalar=rs[:], in1=scale_ap,
                    op0=OP.mult, op1=OP.mult,
                )
                nc.vector.tensor_tensor(out=hb[:], in0=hb[:], in1=bias_ap, op=OP.add)

            def _lnproj_block(t, layer):
                """LN h[:,t] (lnp[layer]) -> proj (wcat[layer]) -> kvb/qsb/xr."""
                hb = sp.tile([P, D], dt.bfloat16, tag="hb")
                _ln_to(hb, t, lnp[:, layer, 0:D], lnp[:, layer, D : 2 * D],
                       act_sqrt=(layer == 0))
                pT = psA.tile([P, P], dt.bfloat16, tag="pT")
                nc.tensor.transpose(out=pT[:], in_=hb[:], identity=ident[:])
                hnTt = sp.tile([P, D], dt.bfloat16, tag="hnTt")
                nc.scalar.copy(hnTt[:], pT[:])
                ps = psB.tile([P, 4 * D], dt.float32, tag="ps")
                nc.tensor.matmul(
                    out=ps[:], lhsT=hnTt[:], rhs=wcat[:, layer, :],
                    start=True, stop=True,
                )
                kvq = sp.tile([P, 2 * D], dt.bfloat16, tag="kvq")
                nc.scalar.copy(kvq[:], ps[:, 0 : 2 * D])
                nc.vector.scalar_tensor_tensor(
                    out=qsb[:, t, :], in0=ps[:, 2 * D : 3 * D], scalar=1.0,
                    in1=bcat[:, layer, 0:D], op0=OP.mult, op1=OP.add,
                )
                nc.vector.scalar_tensor_tensor(
                    out=xr[:, t, :], in0=ps[:, 3 * D : 4 * D], scalar=1.0,
                    in1=bcat[:, layer, D : 2 * D], op0=OP.mult, op1=OP.add,
                )
                nc.scalar.dma_start(kvb.ap()[t * P : (t + 1) * P, :], kvq[:])
                if t + 1 in CH_BLK:
                    ch = CH_BLK.index(t + 1) - 1
                    r0, r1 = CH_ROW[ch], CH_ROW[ch + 1]
                    kvf_l = kvfs[layer % 2]
                    if PROBE_NO_COLLECTIVE:
                        nc.gpsimd.dma_start(
                            out=kvf_l.ap()[8 * r0 : 8 * r0 + (r1 - r0), :],
                            in_=kvb.ap()[r0:r1, :],
                        )
                    else:
                        nc.gpsimd.collective_compute(
                            "AllGather",
                            OP.bypass,
                            replica_groups=[list(range(NCORES))],
                            ins=[kvb.ap()[r0:r1, :].opt()],
                            outs=[kvf_l.ap()[8 * r0 : 8 * r1, :].opt()],
                        )

            def _final_block(t):
                """Final LN on h[:,t] -> out DMA."""
                ot = sp.tile([P, D], dt.float32, tag="ot")
                stats = sp.tile([P, 6], dt.float32, tag="bst")
                nc.vector.bn_stats(stats[:], h[:, t, :])
                mv = sp.tile([P, 2], dt.float32, tag="mv")
                nc.vector.bn_aggr(mv[:], stats[:])
                ve = sp.tile([P, 1], dt.float32, tag="ve")
                nc.vector.tensor_scalar_add(ve[:], mv[:, 1:2], 1e-5)
                rs = sp.tile([P, 1], dt.float32, tag="rs")
                _rsqrt(rs, ve)
                nc.vector.tensor_tensor(
                    out=ot[:], in0=h[:, t, :],
                    in1=mv[:, 0:1].to_broadcast([P, D]), op=OP.subtract,
                )
                nc.vector.scalar_tensor_tensor(
                    out=ot[:], in0=ot[:], scalar=rs[:], in1=fnp[:, 0:D],
                    op0=OP.mult, op1=OP.mult,
                )
                nc.vector.tensor_tensor(
                    out=ot[:], in0=ot[:], in1=fnp[:, D : 2 * D], op=OP.add
                )
                lo = t * P
                hi = min((t + 1) * P, NSH)
                if hi > lo:
                    nc.scalar.dma_start(out_p.ap()[lo:hi, :], ot[0 : hi - lo, :])

            # zero the kvg pool buffers once (trimmed gathers leave stale
            # bytes behind; first use must not see NaN bit patterns)
            for _ in range(GPBUFS):
                z = gp.tile([P, trun, 2 * D], dt.bfloat16, tag="kvg")
                nc.vector.memset(z[:], 0.0)

            # ---- phase 0 (fused with layer-0 LN+proj):
            # h = x @ W_in + b_in + emb_in[idg] + emb_out[odg], the embedding
            # gathers expressed as one-hot matmuls accumulated in PSUM
            for t in range(NBLK):
                ph = psB.tile([P, 4 * D], dt.float32, tag="ps")
                nc.tensor.matmul(out=ph[:, 0:D], lhsT=xTt[:, t, :], rhs=win[:], start=True, stop=False)
                nc.tensor.matmul(
                    out=ph[:, 0:D], lhsT=deg[:, t, 0:P], rhs=eio[:, 0:D],
                    start=False, stop=False,
                )
                nc.tensor.matmul(
                    out=ph[:, 0:D], lhsT=deg[:, t, P : 2 * P], rhs=eio[:, D : 2 * D],
                    start=False, stop=True,
                )
                nc.vector.scalar_tensor_tensor(
                    out=h[:, t, :], in0=ph[:, 0:D], scalar=1.0, in1=bin_t[:],
                    op0=OP.mult, op1=OP.add,
                )
                _lnproj_block(t, 0)

            # ---- layers ----------------------------------------------------
            for layer in range(L):
                kvf = kvfs[layer % 2]
                # ---- edge phase: per (tgt block, bucket) run of trun tiles
                for blk in range(NBLK):
                    pm = psC.tile([P, D + H], dt.float32, tag="pm")
                    for b in range(2):
                        off = (blk * 2 + b) * EDB
                        kit = gp.tile([P, KIB], dt.uint8, tag="kit")
                        nc.sync.dma_start(kit[:], ed_p.ap()[:, off : off + KIB])
                        ed = gp.tile([P, 2 * nrun], dt.uint8, tag="ed")
                        nc.sync.dma_start(ed[:], ed_p.ap()[:, off + KIB : off + EDB])
                        ki = kit[:].bitcast(dt.int16)               # [P, trun*8]
                        sgv = ed[:, 0:nrun].bitcast(dt.float8e4).rearrange(
                            "p (t e) -> p t e", e=P
                        )
                        stv = ed[:, nrun : 2 * nrun].bitcast(dt.float8e4).rearrange(
                            "p (t e) -> p t e", e=P
                        )

                        if ABL_NOEDGE:
                            ue0 = ep.tile([P, trun, D + H], dt.bfloat16, tag="ue")
                            nc.vector.memset(ue0[:], 0.5)
                            for tt in range(trun):
                                nc.tensor.matmul(
                                    out=pm[:], lhsT=sgv[:, tt, :], rhs=ue0[:, tt, :],
                                    start=(b == 0 and tt == 0),
                                    stop=(b == 1 and tt == trun - 1),
                                )
                            continue
                        kvg = gp.tile([P, trun, 2 * D], dt.bfloat16, tag="kvg")
                        for ci, i0 in enumerate(range(0, trun, GBATCH)):
                            i1 = min(i0 + GBATCH, trun)
                            nidx = (i1 - i0) * P
                            if min_run >= i1 * P:
                                creg = nidx  # window always full: static count
                            else:
                                gj = (blk * 2 + b) * NC_G + ci
                                creg = gregs[gregc[0] % len(gregs)]
                                gregc[0] += 1
                                nc.gpsimd.reg_load(creg, gct[0:1, gj : gj + 1])
                            nc.gpsimd.dma_gather(
                                out_ap=kvg[:, i0:i1, :],
                                in_ap=kvf.ap()[b * NBUK0 : b * NBUK0 + (NBUK1 if b else NBUK0), :],
                                idxs_ap=ki[:, i0 * 8 : i1 * 8],
                                num_idxs=nidx, num_idxs_reg=creg,
                                elem_size=2 * D,
                                queue_num=_nextq(),
                            )
                        # q-broadcast via PE: qg[e, f] = q[tl(e), f]
                        qg = ep.tile([P, trun, D], dt.bfloat16, tag="qg")
                        for c0 in range(0, nrun, 512):
                            c1 = min(c0 + 512, nrun)
                            qp = psQ.tile([P, 512], dt.float32, tag="qp")
                            for tt in range(c0 // P, c1 // P):
                                o = tt * P - c0
                                nc.tensor.matmul(
                                    out=qp[:, o : o + P], lhsT=stv[:, tt, :],
                                    rhs=qsb[:, blk, :], start=True, stop=True,
                                )
                            nc.scalar.copy(
                                qg[:, c0 // P : c1 // P, :],
                                qp[:, 0 : c1 - c0].rearrange("p (t e) -> p t e", e=P),
                            )
                        # per-edge logits: alpha = sum_c q*k (tree reduce)
                        qk = ep.tile([P, trun, H, C], dt.bfloat16, tag="qk")
                        nc.vector.tensor_tensor(
                            out=qk[:].rearrange("p t h c -> p t (h c)"),
                            in0=qg[:], in1=kvg[:, :, 0:D], op=OP.mult,
                        )
                        t1 = ep.tile([P, trun, H, 4], dt.bfloat16, tag="t1")
                        with nc.allow_low_precision(reason="alpha logits are O(0.1)"):
                            nc.vector.tensor_tensor(
                                out=t1[:], in0=qk[:, :, :, 0:4], in1=qk[:, :, :, 4:8],
                                op=OP.add,
                            )
                            t2 = ep.tile([P, trun, H, 2], dt.bfloat16, tag="t2")
                            nc.vector.tensor_tensor(
                                out=t2[:], in0=t1[:, :, :, 0:2], in1=t1[:, :, :, 2:4],
                                op=OP.add,
                            )
                            al = ep.tile([P, trun, H, 1], dt.bfloat16, tag="al")
                            nc.vector.tensor_tensor(
                                out=al[:], in0=t2[:, :, :, 0:1], in1=t2[:, :, :, 1:2],
                                op=OP.add,
                            )
                        ue = ep.tile([P, trun, D + H], dt.bfloat16, tag="ue")
                        nc.scalar.activation(
                            out=ue[:, :, D : D + H].rearrange("p t (h o) -> p t h o", o=1),
                            in_=al[:], func=AF.Exp,
                        )
                        wex = ep.tile([P, trun, H, C], dt.bfloat16, tag="wex")
                        nc.scalar.activation(
                            out=wex[:], in_=al[:].to_broadcast([P, trun, H, C]),
                            func=AF.Exp,
                        )
                        nc.vector.tensor_tensor(
                            out=ue[:, :, 0:D], in0=kvg[:, :, D : 2 * D],
                            in1=wex[:].rearrange("p t h c -> p t (h c)"), op=OP.mult,
                        )
                        for tt in range(trun):
                            nc.tensor.matmul(
                                out=pm[:], lhsT=sgv[:, tt, :], rhs=ue[:, tt, :],
                                start=(b == 0 and tt == 0),
                                stop=(b == 1 and tt == trun - 1),
                            )

                    # ---- fused per-block tail: normalize, gate, residual,
                    # then next layer's LN+projection (or final LN) ----------
                    msgb = sp.tile([P, D], dt.float32, tag="msgb")
                    rden = sp.tile([P, H, 1], dt.float32, tag="rden")
                    nc.vector.tensor_scalar_add(
                        rden[:], pm[:, D : D + H].rearrange("p (h o) -> p h o", o=1), 1e-20
                    )
                    nc.vector.reciprocal(rden[:], rden[:])
                    nc.vector.tensor_tensor(
                        out=msgb[:].rearrange("p (h c) -> p h c", c=C),
                        in0=pm[:, 0:D].rearrange("p (h c) -> p h c", c=C),
                        in1=rden[:].to_broadcast([P, H, C]),
                        op=OP.mult,
                    )
                    nc.vector.tensor_tensor(
                        out=msgb[:], in0=msgb[:], in1=bvt[:, layer, :], op=OP.add
                    )
                    scr = sp.tile([P, D], dt.float32, tag="scr")
                    bs1 = sp.tile([P, 1], dt.float32, tag="bs1")
                    nc.vector.scalar_tensor_tensor(
                        out=scr[:], in0=msgb[:], scalar=1.0, in1=wb[:, layer, 0:D],
                        op0=OP.mult, op1=OP.mult, accum_out=bs1[:],
                    )
                    bs2 = sp.tile([P, 1], dt.float32, tag="bs2")
                    nc.vector.scalar_tensor_tensor(
                        out=scr[:], in0=xr[:, blk, :], scalar=1.0, in1=wb[:, layer, D : 2 * D],
                        op0=OP.mult, op1=OP.mult, accum_out=bs2[:],
                    )
                    nc.vector.tensor_tensor(out=bs1[:], in0=bs1[:], in1=bs2[:], op=OP.add)
                    beta = sp.tile([P, 1], dt.float32, tag="beta")
                    nc.scalar.activation(out=beta[:], in_=bs1[:], func=AF.Exp, scale=-1.0)
                    nc.vector.tensor_scalar_add(beta[:], beta[:], 1.0)
                    nc.vector.reciprocal(beta[:], beta[:])
                    # h += msg + beta*(xr - msg)
                    tmpb = sp.tile([P, D], dt.float32, tag="tmpb")
                    nc.vector.tensor_tensor(
                        out=tmpb[:], in0=xr[:, blk, :], in1=msgb[:], op=OP.subtract
                    )
                    nc.vector.scalar_tensor_tensor(
                        out=tmpb[:], in0=tmpb[:], scalar=beta[:], in1=msgb[:],
                        op0=OP.mult, op1=OP.add,
                    )
                    nc.vector.tensor_tensor(
                        out=h[:, blk, :], in0=h[:, blk, :], in1=tmpb[:], op=OP.add
                    )
                    if layer == L - 1:
                        _final_block(blk)
                    else:
                        _lnproj_block(blk, layer + 1)

    nc.finalize()
    return nc

LAST_RES = None


def _make_in_maps(inputs, cores):
    sq = 1.0 / np.sqrt(np.float32(C))
    Wq, Wk, Wv, Wsk = (np.asarray(inputs[k], dtype=np.float32) for k in ("Wq", "Wk", "Wv", "Wskip"))
    bq, bv, bsk = (np.asarray(inputs[k], dtype=np.float32) for k in ("bq", "bv", "bskip"))
    # order per layer: k | v | q*sq | skip  (k-bias dropped: softmax shift
    # invariance; v-bias folded in post-aggregation)
    wcat = np.concatenate([Wk, Wv, Wq * sq, Wsk], axis=2).transpose(1, 0, 2).reshape(D, L * 4 * D)
    bcat = np.concatenate([bq * sq, bsk], axis=1)  # [L, 2D]
    bcat_rep = np.broadcast_to(bcat[:, None, :], (L, P, 2 * D)).transpose(1, 0, 2).reshape(P, L * 2 * D).copy()
    bvp = np.broadcast_to(bv[:, None, :], (L, P, D)).transpose(1, 0, 2).reshape(P, L * D).copy()
    lns, lnb = np.asarray(inputs["ln_scale"], np.float32), np.asarray(inputs["ln_bias"], np.float32)
    lnp = np.broadcast_to(
        np.concatenate([lns, lnb], axis=1)[:, None, :], (L, P, 2 * D)
    ).transpose(1, 0, 2).reshape(P, L * 2 * D).copy()
    fnp = np.broadcast_to(
        np.concatenate([inputs["fn_scale"], inputs["fn_bias"]])[None, :], (P, 2 * D)
    ).astype(np.float32).copy()
    Wbeta = np.asarray(inputs["Wbeta"], np.float32)  # [L, 3D, 1]
    wa = Wbeta[:, 0:D, 0] + Wbeta[:, 2 * D : 3 * D, 0]      # msg coeff
    wbx = Wbeta[:, D : 2 * D, 0] - Wbeta[:, 2 * D : 3 * D, 0]  # xr coeff
    wbeta_rep = np.broadcast_to(
        np.concatenate([wa, wbx], axis=1)[:, None, :], (L, P, 2 * D)
    ).transpose(1, 0, 2).reshape(P, L * 2 * D).copy()
    bin_rep = np.broadcast_to(
        np.asarray(inputs["b_in"], np.float32)[None, :], (P, D)
    ).copy()

    common = dict(
        win=_bf16(inputs["W_in"]),
        bin=bin_rep,
        wcat=_bf16(wcat),
        bcat=bcat_rep,
        bvp=bvp,
        lnp=lnp,
        fnp=fnp,
        wbeta=wbeta_rep,
    )
    KROWS = cores[0]["degoh"].shape[0]
    embio = np.concatenate(
        [np.asarray(inputs["in_emb"], np.float32)[0:KROWS],
         np.asarray(inputs["out_emb"], np.float32)[0:KROWS]], axis=1
    )
    common["embio"] = _bf16(embio)
    in_maps = []
    for c in range(NCORES):
        m = dict(common)
        cd = cores[c]
        m.update(x=cd["x"], edata=cd["edata"], gcnt=cd["gcnt"], degoh=cd["degoh"])
        in_maps.append(m)
    return in_maps


def kernel(**inputs):
    import os

    from concourse.bass_utils import run_bass_kernel_spmd

    x = np.asarray(inputs["x"], dtype=np.float32)
    edge_index = np.asarray(inputs["edge_index"])
    cores, trun, EDB, min_run, KROWS = _preprocess(x, edge_index)
    in_maps = _make_in_maps(inputs, cores)

    nc = _build(trun, EDB, min_run, KROWS)
    kw = {}
    td = os.environ.get("BASS_KERNEL_TMPDIR")
    if td:
        kw["tmpdir"] = td
    res = run_bass_kernel_spmd(nc, in_maps, core_ids=list(range(NCORES)), **kw)
    global LAST_RES
    LAST_RES = res
    outs = [np.asarray(r["out"], dtype=np.float32) for r in res.results]
    return np.concatenate(outs, axis=0)


if __name__ == "__main__":
    import reference

    inp = {k: np.asarray(v) for k, v in reference.setup_inputs().items()}
    exp = np.asarray(reference.reference(**inp))
    act = kernel(**inp)
    err = np.abs(act - exp).max() / (np.abs(exp).max() + 1e-9)
    print("Relative error:", err)
